# revision 10
# baseline (speedup 1.0000x reference)
"""ExpressionBert Trainium2 kernel (v2).

Data-parallel over batch: 8 batch elements -> 8 NeuronCores, no collectives.
Per core: 512 tokens through 6 post-LN transformer layers with
relative_key_query attention.

v2 changes vs baseline:
  - bf16 matmul operands everywhere (weights host-converted, activations
    evicted from PSUM as bf16).  PSUM accumulation and the residual
    stream stay f32.
  - LN affine (g,b) of the preceding LN is folded into Wq/Wk/Wv/W1 on the
    host, so the PE-critical path needs only the normalized xn, not the
    affine output.  The affine is applied off-critical-path for the
    residual stream only.
  - Host-permuted weight layouts -> one contiguous DMA per weight matrix
    per layer (W1/W2 in 4 chunks), issued on the GpSimd SWDGE queue so
    they don't head-of-line block the skew DMAs on the sync queue.
  - Rel-position tables: 4 bands are copied into one flat SBUF tile and
    skewed with a single 3D-AP DMA per (head, side) - 24 skew DMAs/layer
    instead of 96.  k-side bands/skews in bf16 (DVE-copied), q-side stays
    f32 because its skew feeds PE transpose-accumulation into the score
    PSUM.
  - 1/Z partition-broadcast via a PE matmul (ones outer product) instead
    of a DRAM round trip.
"""

import numpy as np
import ml_dtypes

import bass_rust
import concourse.bass as bass
import concourse.mybir as mybir
from concourse import bass_utils
from concourse import tile as tile_mod

f32 = mybir.dt.float32
f32r = mybir.dt.float32r
bf16 = mybir.dt.bfloat16
AF = mybir.ActivationFunctionType
ALU = mybir.AluOpType

# ---- walrus workaround: only ONE sem wait per instruction is supported ----


def _split_multi_waits(nc):
    for f in nc.m.functions:
        for bb in f.blocks:
            new = []
            dirty = False
            for ins in bb.instructions:
                si = ins.sync_info
                if si is not None and len(si.on_wait) > 1:
                    waits = list(si.on_wait)
                    for w in waits[:-1]:
                        nop = mybir.InstNoOp(
                            name=f"waitnop-{nc.next_id()}", ins=[], outs=[])
                        nop.engine = ins.engine
                        nop.sync_info = bass_rust.SyncInfo(
                            on_wait=[w], on_update=[])
                        new.append(nop)
                    ins.sync_info = bass_rust.SyncInfo(
                        on_wait=[waits[-1]], on_update=list(si.on_update))
                    dirty = True
                new.append(ins)
            if dirty:
                bb.instructions = new


class TileContext(tile_mod.TileContext):
    def __exit__(self, exc_type, exc_value, traceback):
        r = super().__exit__(exc_type, exc_value, traceback)
        if exc_type is None:
            _split_multi_waits(self.nc)
        return r


# ---- model dims ----
B, S, F, D, L, H, I = 8, 512, 5, 768, 6, 12, 3072
DH = 64              # head dim
KD = 6               # D / 128
KI = 24              # I / 128
NT = 4               # S / 128
C = 1023             # 2M-1 relative positions
BAND = 640           # per-chunk table band width (639 used + 1 pad)
SCALE = 1.0 / np.sqrt(DH)
EPS = 1e-12

_CACHED = {}


def build_module():
    nc = bass.Bass()

    # ---------------- DRAM I/O ----------------
    xT = nc.dram_tensor("xT", [F, S], bf16, kind="ExternalInput")
    mask_col = nc.dram_tensor("mask_col", [S, 1], f32, kind="ExternalInput")
    inw = nc.dram_tensor("inw", [F, D], bf16, kind="ExternalInput")
    ttib = nc.dram_tensor("ttib", [D], bf16, kind="ExternalInput")
    embgb = nc.dram_tensor("embgb", [2 * D], bf16, kind="ExternalInput")
    wq_p = nc.dram_tensor("wq_p", [L, 128, KD * KD * 128], bf16,
                          kind="ExternalInput")
    wk_p = nc.dram_tensor("wk_p", [L, 128, KD * KD * 128], bf16,
                          kind="ExternalInput")
    wv_p = nc.dram_tensor("wv_p", [L, 128, KD * D], bf16,
                          kind="ExternalInput")
    wo_p = nc.dram_tensor("wo_p", [L, 128, KD * D], bf16,
                          kind="ExternalInput")
    w1_p = nc.dram_tensor("w1_p", [L, 128, KI * KD * 128], bf16,
                          kind="ExternalInput")
    w2_p = nc.dram_tensor("w2_p", [L, 128, KI * D], bf16,
                          kind="ExternalInput")
    bqc = nc.dram_tensor("bqc", [L, KD, 128], f32, kind="ExternalInput")
    bkc = nc.dram_tensor("bkc", [L, KD, 128], f32, kind="ExternalInput")
    b1c = nc.dram_tensor("b1c", [L, KI, 128], f32, kind="ExternalInput")
    lnpack = nc.dram_tensor("lnpack", [L, 7 * D], bf16, kind="ExternalInput")
    de_t = nc.dram_tensor("de_t", [L, DH, C], bf16, kind="ExternalInput")
    de_rt = nc.dram_tensor("de_rt", [L, DH, C], bf16, kind="ExternalInput")
    ident_in = nc.dram_tensor("ident_in", [128, 128], bf16,
                              kind="ExternalInput")
    identr_in = nc.dram_tensor("identr_in", [128, 128], f32,
                               kind="ExternalInput")
    ones_col_in = nc.dram_tensor("ones_col_in", [128, 1], bf16,
                                 kind="ExternalInput")
    zsel_in = nc.dram_tensor("zsel_in", [128, 4], bf16,
                              kind="ExternalInput")
    bsel_in = nc.dram_tensor("bsel_in", [2, 128], f32,
                             kind="ExternalInput")
    y = nc.dram_tensor("y", [S, D], f32, kind="ExternalOutput")

    with TileContext(nc) as tc:
        with tc.tile_pool(name="resid", bufs=1) as p_res, \
             tc.tile_pool(name="fm", bufs=1) as p_fm, \
             tc.tile_pool(name="attn", bufs=1) as p_at, \
             tc.tile_pool(name="wpool", bufs=1) as p_w, \
             tc.tile_pool(name="cpool", bufs=1) as p_c, \
             tc.tile_pool(name="spool", bufs=2) as p_s, \
             tc.tile_pool(name="psum", bufs=1, space="PSUM") as p_ps:

            def acc_tile(i):
                return p_ps.tile([128, 512], f32, tag=f"acc{i}",
                                 name=f"acc{i}")

            def bnd_tile():
                return p_ps.tile([128, 1024], f32, tag="bnd", bufs=2,
                                 name="bnd")

            # ---- constants ----
            ident = p_c.tile([128, 128], bf16, tag="ident", name="ident")
            nc.sync.dma_start(ident[:], ident_in[:])
            ident_r = p_c.tile([128, 128], f32r, tag="identr", name="identr")
            nc.sync.dma_start(ident_r[:], identr_in[:].bitcast(f32r))
            ones_col = p_c.tile([128, 1], bf16, tag="onesc", name="ones_col")
            nc.sync.dma_start(ones_col[:], ones_col_in[:])
            zsel = p_c.tile([128, 4], bf16, tag="zsel", name="zsel")
            nc.sync.dma_start(zsel[:], zsel_in[:])
            bsel = p_c.tile([2, 128], f32r, tag="bsel", name="bsel")
            nc.sync.dma_start(bsel[:], bsel_in[:].bitcast(f32r))
            eps_c = p_c.tile([128, 1], f32, tag="eps", name="eps_c")
            nc.vector.memset(eps_c[:], EPS)
            invd_c = p_c.tile([128, 1], f32, tag="invd", name="invd_c")
            nc.vector.memset(invd_c[:], 1.0 / D)
            masks = []
            for t in range(NT):
                mt = p_c.tile([128, 1], f32, tag=f"mask{t}", name=f"mask{t}")
                nc.sync.dma_start(mt[:], mask_col[t * 128:(t + 1) * 128, :])
                masks.append(mt)

            # ---- LayerNorm: x_t f32 [128,D] -> xn (bf16, normalized) and
            # h_out = xn*g + b (f32 residual). g/b applied only to h_out.
            def layernorm(x_t, xn_out, h_out, g_ap, b_ap):
                sum_ = p_s.tile([128, 1], f32, tag="sum", name="sum")
                nc.vector.tensor_reduce(out=sum_[:], in_=x_t[:],
                                        axis=mybir.AxisListType.X, op=ALU.add)
                sq = p_s.tile([128, D], f32, tag="hp", name="sq")
                ssq = p_s.tile([128, 1], f32, tag="ssq", name="ssq")
                nc.scalar.activation(sq[:], x_t[:], AF.Square,
                                     accum_out=ssq[:])
                mu = p_s.tile([128, 1], f32, tag="mu", name="mu")
                nc.scalar.mul(mu[:], sum_[:], 1.0 / D)
                s2 = p_s.tile([128, 1], f32, tag="s2", name="s2")
                nc.vector.tensor_mul(s2[:], mu[:], mu[:])
                var = p_s.tile([128, 1], f32, tag="var", name="var")
                nc.vector.scalar_tensor_tensor(
                    out=var[:], in0=ssq[:], scalar=invd_c[:], in1=s2[:],
                    op0=ALU.mult, op1=ALU.subtract)
                std = p_s.tile([128, 1], f32, tag="std", name="std")
                nc.scalar.activation(std[:], var[:], AF.Sqrt, bias=eps_c[:])
                rstd = p_s.tile([128, 1], f32, tag="rstd", name="rstd")
                nc.vector.reciprocal(rstd[:], std[:])
                nc.vector.scalar_tensor_tensor(
                    out=xn_out[:], in0=x_t[:], scalar=mu[:],
                    in1=rstd[:].to_broadcast((128, D)),
                    op0=ALU.subtract, op1=ALU.mult)
                tmp = p_s.tile([128, D], f32, tag="hp", name="lntmp")
                nc.vector.tensor_mul(tmp[:], xn_out[:], g_ap)
                nc.vector.tensor_add(h_out[:], tmp[:], b_ap)

            # ---- embedding ----
            xT_sb = p_c.tile([F, S], bf16, tag="xT", name="xT_sb")
            nc.sync.dma_start(xT_sb[:], xT[:])
            inw_sb = p_c.tile([F, D], bf16, tag="inw", name="inw_sb")
            nc.sync.dma_start(inw_sb[:], inw[:])
            ttib_bc = p_c.tile([128, D], bf16, tag="ttib", name="ttib_bc")
            nc.sync.dma_start(ttib_bc[:], bass.AP(
                tensor=ttib, offset=0, ap=[[0, 128], [1, D]]))
            embgb_bc = p_c.tile([128, 7 * D], bf16, tag="lnp",
                                name="embgb_bc")
            nc.sync.dma_start(embgb_bc[:, 0:2 * D], bass.AP(
                tensor=embgb, offset=0, ap=[[0, 128], [1, 2 * D]]))

            h, xn = [], []
            for t in range(NT):
                pe0 = acc_tile(t % 4)
                nc.tensor.matmul(pe0[:, 0:512],
                                 xT_sb[:, t * 128:(t + 1) * 128],
                                 inw_sb[:, 0:512], start=True, stop=True)
                pe1 = bnd_tile()
                nc.tensor.matmul(pe1[:, 0:256],
                                 xT_sb[:, t * 128:(t + 1) * 128],
                                 inw_sb[:, 512:768], start=True, stop=True)
                he = p_s.tile([128, D], f32, tag="hp", name="he")
                nc.vector.tensor_add(he[:, 0:512], pe0[:, 0:512],
                                     ttib_bc[:, 0:512])
                nc.vector.tensor_add(he[:, 512:768], pe1[:, 0:256],
                                     ttib_bc[:, 512:768])
                xnt = p_res.tile([128, D], f32r, tag=f"xn{t}", name=f"xn{t}")
                ht = p_res.tile([128, D], f32, tag=f"h{t}", name=f"h{t}")
                layernorm(he, xnt, ht, embgb_bc[:, 0:D], embgb_bc[:, D:2 * D])
                h.append(ht)
                xn.append(xnt)

            # ================= layers =================
            for l in range(L):
                # ---- weight / table / bias loads ----
                wq_sb = p_w.tile([128, KD * KD * 128], bf16, tag="wq",
                                 name="wq_sb")
                nc.gpsimd.dma_start(wq_sb[:], wq_p[l])
                wk_sb = p_w.tile([128, KD * KD * 128], bf16, tag="wk",
                                 name="wk_sb")
                nc.gpsimd.dma_start(wk_sb[:], wk_p[l])
                wv_sb = p_w.tile([128, KD * D], bf16, tag="wv", name="wv_sb")
                nc.gpsimd.dma_start(wv_sb[:], wv_p[l])
                wo_sb = p_w.tile([128, KD * D], bf16, tag="wo", name="wo_sb")
                nc.gpsimd.dma_start(wo_sb[:], wo_p[l])
                det_sb = p_c.tile([128, C + 1], bf16, tag="det",
                                  name="det_sb")
                nc.gpsimd.dma_start(det_sb[0:DH, 0:C], de_t[l])
                nc.gpsimd.dma_start(det_sb[DH:128, 0:C], de_t[l])
                dert_sb = p_c.tile([128, C + 1], bf16, tag="dert",
                                   name="dert_sb")
                nc.gpsimd.dma_start(dert_sb[0:DH, 0:C], de_rt[l])
                nc.gpsimd.dma_start(dert_sb[DH:128, 0:C], de_rt[l])
                lnp = p_c.tile([128, 7 * D], bf16, tag="lnp", name="lnp")
                nc.sync.dma_start(lnp[:], bass.AP(
                    tensor=lnpack, offset=l * 7 * D,
                    ap=[[0, 128], [1, 7 * D]]))
                bq_sb = p_c.tile([128, KD], f32, tag="bqc", name="bq_sb")
                nc.sync.dma_start(bq_sb[:], bass.AP(
                    tensor=bqc, offset=l * KD * 128,
                    ap=[[1, 128], [128, KD]]))
                bk_sb = p_c.tile([128, KD], f32, tag="bkc", name="bk_sb")
                nc.sync.dma_start(bk_sb[:], bass.AP(
                    tensor=bkc, offset=l * KD * 128,
                    ap=[[1, 128], [128, KD]]))
                b1_sb = p_c.tile([128, KI], f32, tag="b1c", name="b1_sb")
                nc.sync.dma_start(b1_sb[:], bass.AP(
                    tensor=b1c, offset=l * KI * 128,
                    ap=[[1, 128], [128, KI]]))
                bv_bc = lnp[:, 0 * D:1 * D]
                bo_bc = lnp[:, 1 * D:2 * D]
                b2_bc = lnp[:, 2 * D:3 * D]
                g1_bc = lnp[:, 3 * D:4 * D]
                be1_bc = lnp[:, 4 * D:5 * D]
                g2_bc = lnp[:, 5 * D:6 * D]
                be2_bc = lnp[:, 6 * D:7 * D]

                # ---- phase A: h_T feature-major via PE transposes of xn ----
                h_T = []
                for k in range(KD):
                    tp = acc_tile(k % 4).bitcast(f32r)
                    for t in range(NT):
                        nc.tensor.matmul(
                            tp[:, t * 128:(t + 1) * 128],
                            xn[t][:, k * 128:(k + 1) * 128], ident_r[:],
                            is_transpose=True, start=True, stop=True)
                    hTk = p_fm.tile([128, S], bf16, tag=f"hT{k}",
                                    name=f"hT{k}")
                    nc.scalar.activation(hTk[:], tp[:], AF.Copy)
                    h_T.append(hTk)

                # ---- phase B: Q^T, K^T feature-major ----
                q_T, k_T = [], []
                for e in range(KD):
                    psq = acc_tile(e % 2)
                    psk = acc_tile(2 + e % 2)
                    for k in range(KD):
                        co = (e * KD + k) * 128
                        nc.tensor.matmul(psq[:], wq_sb[:, co:co + 128],
                                         h_T[k][:],
                                         start=(k == 0), stop=(k == KD - 1))
                        nc.tensor.matmul(psk[:], wk_sb[:, co:co + 128],
                                         h_T[k][:],
                                         start=(k == 0), stop=(k == KD - 1))
                    qT = p_fm.tile([128, S], bf16, tag=f"qT{e}",
                                   name=f"qT{e}")
                    nc.scalar.activation(qT[:], psq[:], AF.Identity,
                                         bias=bq_sb[:, e:e + 1])
                    kT = p_fm.tile([128, S], bf16, tag=f"kT{e}",
                                   name=f"kT{e}")
                    nc.scalar.activation(kT[:], psk[:], AF.Identity,
                                         bias=bk_sb[:, e:e + 1])
                    q_T.append(qT)
                    k_T.append(kT)

                # ---- V token-major ----
                V = []
                for t in range(NT):
                    V.append(p_fm.tile([128, D], bf16, tag=f"V{t}",
                                       name=f"V{t}"))
                for half in range(2):
                    ts = (2 * half, 2 * half + 1)
                    pss = {}
                    for ti, t in enumerate(ts):
                        pss[(t, 0)] = acc_tile(2 * ti)
                        pss[(t, 1)] = acc_tile(2 * ti + 1)
                    for k in range(KD):
                        for t in ts:
                            nc.tensor.matmul(
                                pss[(t, 0)][:, 0:384],
                                h_T[k][:, t * 128:(t + 1) * 128],
                                wv_sb[:, k * D:k * D + 384],
                                start=(k == 0), stop=(k == KD - 1))
                            nc.tensor.matmul(
                                pss[(t, 1)][:, 0:384],
                                h_T[k][:, t * 128:(t + 1) * 128],
                                wv_sb[:, k * D + 384:k * D + 768],
                                start=(k == 0), stop=(k == KD - 1))
                    for t in ts:
                        nc.vector.tensor_add(V[t][:, 0:384],
                                             pss[(t, 0)][:, 0:384],
                                             bv_bc[:, 0:384])
                        nc.vector.tensor_add(V[t][:, 384:768],
                                             pss[(t, 1)][:, 0:384],
                                             bv_bc[:, 384:768])

                # ---- phase C: attention, one head pair per etile ----
                # C1: table bands + skew DMAs for both heads (q/k bands
                # interleaved so ACT and DVE copies run in parallel).
                # C2: scores/softmax/AV per head; skew latency of head r
                # hides under C1 of head r+1 / C2 of head r-1.
                ctx_T = []
                for e in range(KD):
                    skq, skk = [], []
                    for r in range(2):
                        qh = q_T[e][64 * r:64 * r + 64, :]
                        kh = k_T[e][64 * r:64 * r + 64, :]
                        dlo, dhi = 64 * r, 64 * r + 64
                        qba = p_at.tile([128, 4 * BAND], f32r, tag="qba",
                                        name="qba")
                        kba = p_at.tile([128, 4 * BAND], bf16, tag="kba",
                                        name="kba")
                        for qt in range(NT):
                            bs = 384 - 128 * qt
                            bnd = bnd_tile()
                            nc.tensor.matmul(
                                bnd[:, 0:512],
                                qh[:, qt * 128:(qt + 1) * 128],
                                dert_sb[dlo:dhi, bs:bs + 512],
                                start=True, stop=True)
                            nc.tensor.matmul(
                                bnd[:, 512:640],
                                qh[:, qt * 128:(qt + 1) * 128],
                                dert_sb[dlo:dhi, bs + 512:bs + 640],
                                start=True, stop=True)
                            nc.scalar.activation(
                                qba[:, qt * BAND:qt * BAND + 640],
                                bnd[:, 0:640], AF.Copy)
                            bnd = bnd_tile()
                            nc.tensor.matmul(
                                bnd[:, 0:512],
                                kh[:, qt * 128:(qt + 1) * 128],
                                det_sb[dlo:dhi, bs:bs + 512],
                                start=True, stop=True)
                            nc.tensor.matmul(
                                bnd[:, 512:640],
                                kh[:, qt * 128:(qt + 1) * 128],
                                det_sb[dlo:dhi, bs + 512:bs + 640],
                                start=True, stop=True)
                            nc.vector.tensor_copy(
                                kba[:, qt * BAND:qt * BAND + 640],
                                bnd[:, 0:640])
                        s2q = p_at.tile([128, 4 * S], f32r, tag="s2q",
                                        name="s2q")
                        nc.sync.dma_start(s2q[:], bass.AP(
                            tensor=qba.tensor, offset=qba.offset + 127,
                            ap=[[4 * BAND - 1, 128], [BAND, 4], [1, S]]))
                        s3t = p_at.tile([128, 4 * S], bf16, tag="s3t",
                                        name="s3t")
                        nc.sync.dma_start(s3t[:], bass.AP(
                            tensor=kba.tensor, offset=kba.offset + 127,
                            ap=[[4 * BAND - 1, 128], [BAND, 4], [1, S]]))
                        skq.append(s2q)
                        skk.append(s3t)

                    psc = None
                    z01 = None
                    for r in range(2):
                        qh = q_T[e][64 * r:64 * r + 64, :]
                        kh = k_T[e][64 * r:64 * r + 64, :]
                        st = [acc_tile(i) for i in range(4)]
                        for kt in range(4):
                            nc.tensor.matmul(
                                st[kt][:], kh[:, kt * 128:(kt + 1) * 128],
                                qh[:], start=True, stop=False)
                        # transpose-accumulate q-side into S^T
                        for qt in range(NT):
                            for kt in range(4):
                                nc.tensor.matmul(
                                    st[kt][:, qt * 128:(qt + 1) * 128]
                                    .bitcast(f32r),
                                    skq[r][:, qt * S + kt * 128:
                                           qt * S + (kt + 1) * 128],
                                    ident_r[:], is_transpose=True,
                                    start=False, stop=(qt == NT - 1))
                        # k-side add + exp
                        pTs = []
                        for kt in range(4):
                            nc.vector.tensor_add(
                                st[kt][:], st[kt][:],
                                skk[r][:, kt * S:(kt + 1) * S])
                            pT = p_at.tile([128, S], bf16, tag="pT",
                                           bufs=4, name="pT")
                            nc.scalar.activation(pT[:], st[kt][:], AF.Exp,
                                                 bias=masks[kt][:],
                                                 scale=float(SCALE))
                            pTs.append(pT)
                        # Z rows for both heads accumulate into z01[0:2]
                        if r == 0:
                            z01 = bnd_tile()
                        for kt in range(4):
                            nc.tensor.matmul(
                                z01[0:2, 0:512], zsel[:, 2 * r:2 * r + 2],
                                pTs[kt][:],
                                start=(r == 0 and kt == 0),
                                stop=(r == 1 and kt == 3))
                        # AV for this head into shared psc
                        if r == 0:
                            psc = bnd_tile()
                        hh = 2 * e + r
                        for kt in range(4):
                            nc.tensor.matmul(
                                psc[64 * r:64 * r + 64, 0:512],
                                V[kt][:, hh * 64:hh * 64 + 64],
                                pTs[kt][:],
                                start=(kt == 0), stop=(kt == 3))
                    rz01 = p_at.tile([2, S], f32r, tag="rz01", name="rz01")
                    with nc.allow_low_precision(reason="1/Z in f32r"):
                        nc.vector.reciprocal(rz01[:], z01[0:2, 0:512])
                    rzb = acc_tile(0)
                    nc.tensor.matmul(rzb[:, :], bsel[:], rz01[:],
                                     start=True, stop=True)
                    rzbs = p_at.tile([128, S], bf16, tag="rzs", name="rzbs")
                    nc.scalar.activation(rzbs[:], rzb[:], AF.Copy)
                    cT = p_fm.tile([128, S], bf16, tag=f"qT{e}",
                                   name=f"cT{e}")
                    nc.vector.tensor_mul(cT[:], psc[0:128, 0:512], rzbs[:])
                    ctx_T.append(cT)

                # ---- phase D: O-proj + residual + LN1 ----
                h1, xn1 = [], []
                for half in range(2):
                    ts = (2 * half, 2 * half + 1)
                    pss = {}
                    for ti, t in enumerate(ts):
                        pss[(t, 0)] = acc_tile(2 * ti)
                        pss[(t, 1)] = acc_tile(2 * ti + 1)
                    for k in range(KD):
                        for t in ts:
                            nc.tensor.matmul(
                                pss[(t, 0)][:, 0:384],
                                ctx_T[k][:, t * 128:(t + 1) * 128],
                                wo_sb[:, k * D:k * D + 384],
                                start=(k == 0), stop=(k == KD - 1))
                            nc.tensor.matmul(
                                pss[(t, 1)][:, 0:384],
                                ctx_T[k][:, t * 128:(t + 1) * 128],
                                wo_sb[:, k * D + 384:k * D + 768],
                                start=(k == 0), stop=(k == KD - 1))
                    for t in ts:
                        hp = p_s.tile([128, D], f32, tag="hp", name="hp")
                        nc.vector.tensor_add(hp[:, 0:384],
                                             pss[(t, 0)][:, 0:384],
                                             h[t][:, 0:384])
                        nc.vector.tensor_add(hp[:, 384:768],
                                             pss[(t, 1)][:, 0:384],
                                             h[t][:, 384:768])
                        nc.vector.tensor_add(hp[:], hp[:], bo_bc[:])
                        xnt = p_res.tile([128, D], f32r, tag=f"xn{t}",
                                         name=f"xn1_{t}")
                        h1t = p_res.tile([128, D], f32, tag=f"h1_{t}",
                                         name=f"h1_{t}")
                        layernorm(hp, xnt, h1t, g1_bc, be1_bc)
                        xn1.append(xnt)
                        h1.append(h1t)

                # ---- phase E: FFN ----
                h1_T = []
                for k in range(KD):
                    tp = acc_tile(k % 4).bitcast(f32r)
                    for t in range(NT):
                        nc.tensor.matmul(
                            tp[:, t * 128:(t + 1) * 128],
                            xn1[t][:, k * 128:(k + 1) * 128], ident_r[:],
                            is_transpose=True, start=True, stop=True)
                    hTk = p_fm.tile([128, S], bf16, tag=f"hT{k}",
                                    name=f"h1T{k}")
                    nc.scalar.activation(hTk[:], tp[:], AF.Copy)
                    h1_T.append(hTk)

                for blk in range(4):
                    w1c = p_w.tile([128, 6 * KD * 128], bf16, tag="w1c",
                                   bufs=2, name="w1c")
                    nc.gpsimd.dma_start(
                        w1c[:],
                        w1_p[l, :, blk * 6 * KD * 128:
                             (blk + 1) * 6 * KD * 128])
                    w2c = p_w.tile([128, 6 * D], bf16, tag="w2c",
                                   bufs=2, name="w2c")
                    nc.gpsimd.dma_start(
                        w2c[:], w2_p[l, :, blk * 6 * D:(blk + 1) * 6 * D])
                    g_T = []
                    for j in range(6):
                        i = blk * 6 + j
                        ps = acc_tile(j % 2)
                        for k in range(KD):
                            co = (j * KD + k) * 128
                            nc.tensor.matmul(
                                ps[:], w1c[:, co:co + 128], h1_T[k][:],
                                start=(k == 0), stop=(k == KD - 1))
                        gt = p_fm.tile([128, S], bf16, tag=f"gT{j}",
                                       name=f"gT{j}")
                        nc.scalar.activation(gt[:], ps[:], AF.Gelu,
                                             bias=b1_sb[:, i:i + 1])
                        g_T.append(gt)
                    for half in range(2):
                        ts = (2 * half, 2 * half + 1)
                        pss = {}
                        for ti, t in enumerate(ts):
                            pss[(t, 0)] = acc_tile(2 + ti)
                            pss[(t, 1)] = bnd_tile()
                        for j in range(6):
                            for t in ts:
                                nc.tensor.matmul(
                                    pss[(t, 0)][:, 0:384],
                                    g_T[j][:, t * 128:(t + 1) * 128],
                                    w2c[:, j * D:j * D + 384],
                                    start=(j == 0), stop=(j == 5))
                                nc.tensor.matmul(
                                    pss[(t, 1)][:, 0:384],
                                    g_T[j][:, t * 128:(t + 1) * 128],
                                    w2c[:, j * D + 384:j * D + 768],
                                    start=(j == 0), stop=(j == 5))
                        for t in ts:
                            nc.vector.tensor_add(h1[t][:, 0:384],
                                                 h1[t][:, 0:384],
                                                 pss[(t, 0)][:, 0:384])
                            nc.vector.tensor_add(h1[t][:, 384:768],
                                                 h1[t][:, 384:768],
                                                 pss[(t, 1)][:, 0:384])

                new_h, new_xn = [], []
                for t in range(NT):
                    nc.vector.tensor_add(h1[t][:], h1[t][:], b2_bc[:])
                    xnt = p_res.tile([128, D], f32r, tag=f"xn{t}",
                                     name=f"nxn{t}")
                    ht = p_res.tile([128, D], f32, tag=f"h{t}",
                                    name=f"nh{t}")
                    layernorm(h1[t], xnt, ht, g2_bc, be2_bc)
                    new_h.append(ht)
                    new_xn.append(xnt)
                h, xn = new_h, new_xn

            for t in range(NT):
                nc.sync.dma_start(y[t * 128:(t + 1) * 128, :], h[t][:])

    return nc


def _prep_inputs(inputs):
    fp = np.float32
    b16 = ml_dtypes.bfloat16
    ii = np.ascontiguousarray(inputs["input_ids"], fp)
    am = np.ascontiguousarray(inputs["attn_mask"], fp)
    de = np.asarray(inputs["dist_emb"], fp)
    wq = np.asarray(inputs["wq"], fp)
    wk = np.asarray(inputs["wk"], fp)
    wv = np.asarray(inputs["wv"], fp)
    wo = np.asarray(inputs["wo"], fp)
    w1 = np.asarray(inputs["w1"], fp)
    w2 = np.asarray(inputs["w2"], fp)
    bq = np.asarray(inputs["bq"], fp)
    bk = np.asarray(inputs["bk"], fp)
    bv = np.asarray(inputs["bv"], fp)
    bo = np.asarray(inputs["bo"], fp)
    b1 = np.asarray(inputs["b1"], fp)
    b2 = np.asarray(inputs["b2"], fp)
    g1 = np.asarray(inputs["ln1_g"], fp)
    be1 = np.asarray(inputs["ln1_b"], fp)
    g2 = np.asarray(inputs["ln2_g"], fp)
    be2 = np.asarray(inputs["ln2_b"], fp)
    emb_g = np.asarray(inputs["emb_ln_g"], fp)
    emb_b = np.asarray(inputs["emb_ln_b"], fp)

    # fold the preceding LN affine into Wq/Wk/Wv; fold ln1 into W1
    gprev = np.stack([emb_g] + [g2[ll] for ll in range(L - 1)])
    bprev = np.stack([emb_b] + [be2[ll] for ll in range(L - 1)])
    wqf = gprev[:, :, None] * wq
    wkf = gprev[:, :, None] * wk
    wvf = gprev[:, :, None] * wv
    bqf = bq + np.einsum('ld,lde->le', bprev, wq)
    bkf = bk + np.einsum('ld,lde->le', bprev, wk)
    bvf = bv + np.einsum('ld,lde->le', bprev, wv)
    w1f = g1[:, :, None] * w1
    b1f = b1 + np.einsum('ld,ldi->li', be1, w1)

    def cb(x):
        return np.ascontiguousarray(x.astype(b16))

    shared = dict(
        inw=cb(np.asarray(inputs["in_w"], fp)),
        ttib=cb(np.asarray(inputs["in_b"] + inputs["tte"], fp)),
        embgb=cb(np.concatenate([emb_g, emb_b])),
        wq_p=cb(wqf.reshape(L, KD, 128, KD, 128)
                .transpose(0, 2, 3, 1, 4).reshape(L, 128, KD * KD * 128)),
        wk_p=cb(wkf.reshape(L, KD, 128, KD, 128)
                .transpose(0, 2, 3, 1, 4).reshape(L, 128, KD * KD * 128)),
        wv_p=cb(wvf.reshape(L, KD, 128, D)
                .transpose(0, 2, 1, 3).reshape(L, 128, KD * D)),
        wo_p=cb(wo.reshape(L, KD, 128, D)
                .transpose(0, 2, 1, 3).reshape(L, 128, KD * D)),
        w1_p=cb(w1f.reshape(L, KD, 128, KI, 128)
                .transpose(0, 2, 3, 1, 4).reshape(L, 128, KI * KD * 128)),
        w2_p=cb(w2.reshape(L, KI, 128, D)
                .transpose(0, 2, 1, 3).reshape(L, 128, KI * D)),
        bqc=np.ascontiguousarray(bqf.reshape(L, KD, 128), fp),
        bkc=np.ascontiguousarray(bkf.reshape(L, KD, 128), fp),
        b1c=np.ascontiguousarray(b1f.reshape(L, KI, 128), fp),
        lnpack=cb(np.stack([bvf, bo, b2, g1, be1, g2, be2], axis=1)
                  .reshape(L, 7 * D)),
        de_t=cb(de.transpose(0, 2, 1)),
        de_rt=cb(de[:, ::-1, :].transpose(0, 2, 1)),
        ident_in=cb(np.eye(128, dtype=fp)),
        identr_in=np.eye(128, dtype=fp),
        ones_col_in=cb(np.ones((128, 1), fp)),
        zsel_in=cb(np.stack([np.ones(128), np.zeros(128),
                             np.zeros(128), np.ones(128)], axis=1)),
        bsel_in=np.ascontiguousarray(np.stack([
            np.concatenate([np.ones(64), np.zeros(64)]),
            np.concatenate([np.zeros(64), np.ones(64)])]), fp),
    )
    in_maps = []
    for c in range(B):
        m = dict(shared)
        m["xT"] = cb(ii[c].T)
        m["mask_col"] = np.ascontiguousarray(
            ((1.0 - am[c]) * -1e9)[:, None], fp)
        in_maps.append(m)
    return in_maps


def kernel(trace=False, **inputs):
    if "nc" not in _CACHED:
        _CACHED["nc"] = build_module()
    nc = _CACHED["nc"]
    in_maps = _prep_inputs(inputs)
    res = bass_utils.run_bass_kernel_spmd(
        nc, in_maps, core_ids=list(range(B)), trace=trace)
    out = np.stack([np.asarray(res.results[c]["y"], np.float32)
                    for c in range(B)])
    if trace:
        kernel.last_exec_time_ns = res.exec_time_ns
        kernel.last_results = res
    return out


# revision 12
# speedup vs baseline: 1.0415x; 1.0415x over previous
"""ExpressionBert Trainium2 kernel (v2).

Data-parallel over batch: 8 batch elements -> 8 NeuronCores, no collectives.
Per core: 512 tokens through 6 post-LN transformer layers with
relative_key_query attention.

v2 changes vs baseline:
  - bf16 matmul operands everywhere (weights host-converted, activations
    evicted from PSUM as bf16).  PSUM accumulation and the residual
    stream stay f32.
  - LN affine (g,b) of the preceding LN is folded into Wq/Wk/Wv/W1 on the
    host, so the PE-critical path needs only the normalized xn, not the
    affine output.  The affine is applied off-critical-path for the
    residual stream only.
  - Host-permuted weight layouts -> one contiguous DMA per weight matrix
    per layer (W1/W2 in 4 chunks), issued on the GpSimd SWDGE queue so
    they don't head-of-line block the skew DMAs on the sync queue.
  - Rel-position tables: 4 bands are copied into one flat SBUF tile and
    skewed with a single 3D-AP DMA per (head, side) - 24 skew DMAs/layer
    instead of 96.  k-side bands/skews in bf16 (DVE-copied), q-side stays
    f32 because its skew feeds PE transpose-accumulation into the score
    PSUM.
  - 1/Z partition-broadcast via a PE matmul (ones outer product) instead
    of a DRAM round trip.
"""

import numpy as np
import ml_dtypes

import bass_rust
import concourse.bass as bass
import concourse.mybir as mybir
from concourse import bass_utils
from concourse import tile as tile_mod

f32 = mybir.dt.float32
f32r = mybir.dt.float32r
bf16 = mybir.dt.bfloat16
AF = mybir.ActivationFunctionType
ALU = mybir.AluOpType

# ---- walrus workaround: only ONE sem wait per instruction is supported ----


def _split_multi_waits(nc):
    for f in nc.m.functions:
        for bb in f.blocks:
            new = []
            dirty = False
            for ins in bb.instructions:
                si = ins.sync_info
                if si is not None and len(si.on_wait) > 1:
                    waits = list(si.on_wait)
                    for w in waits[:-1]:
                        nop = mybir.InstNoOp(
                            name=f"waitnop-{nc.next_id()}", ins=[], outs=[])
                        nop.engine = ins.engine
                        nop.sync_info = bass_rust.SyncInfo(
                            on_wait=[w], on_update=[])
                        new.append(nop)
                    ins.sync_info = bass_rust.SyncInfo(
                        on_wait=[waits[-1]], on_update=list(si.on_update))
                    dirty = True
                new.append(ins)
            if dirty:
                bb.instructions = new


class TileContext(tile_mod.TileContext):
    def __exit__(self, exc_type, exc_value, traceback):
        r = super().__exit__(exc_type, exc_value, traceback)
        if exc_type is None:
            _split_multi_waits(self.nc)
        return r


# ---- model dims ----
B, S, F, D, L, H, I = 8, 512, 5, 768, 6, 12, 3072
DH = 64              # head dim
KD = 6               # D / 128
KI = 24              # I / 128
NT = 4               # S / 128
C = 1023             # 2M-1 relative positions
BAND = 640           # per-chunk table band width (639 used + 1 pad)
SCALE = 1.0 / np.sqrt(DH)
EPS = 1e-12

_CACHED = {}


def build_module():
    nc = bass.Bass()

    # ---------------- DRAM I/O ----------------
    xT = nc.dram_tensor("xT", [F, S], bf16, kind="ExternalInput")
    mask_col = nc.dram_tensor("mask_col", [S, 1], f32, kind="ExternalInput")
    inw = nc.dram_tensor("inw", [F, D], bf16, kind="ExternalInput")
    ttib = nc.dram_tensor("ttib", [D], bf16, kind="ExternalInput")
    embgb = nc.dram_tensor("embgb", [2 * D], bf16, kind="ExternalInput")
    wq_p = nc.dram_tensor("wq_p", [L, 128, KD * KD * 128], bf16,
                          kind="ExternalInput")
    wk_p = nc.dram_tensor("wk_p", [L, 128, KD * KD * 128], bf16,
                          kind="ExternalInput")
    wv_p = nc.dram_tensor("wv_p", [L, 128, KD * D], bf16,
                          kind="ExternalInput")
    wo_p = nc.dram_tensor("wo_p", [L, 128, KD * D], bf16,
                          kind="ExternalInput")
    w1_p = nc.dram_tensor("w1_p", [L, 128, KI * KD * 128], bf16,
                          kind="ExternalInput")
    w2_p = nc.dram_tensor("w2_p", [L, 128, KI * D], bf16,
                          kind="ExternalInput")
    bqc = nc.dram_tensor("bqc", [L, KD, 128], f32, kind="ExternalInput")
    bkc = nc.dram_tensor("bkc", [L, KD, 128], f32, kind="ExternalInput")
    b1c = nc.dram_tensor("b1c", [L, KI, 128], f32, kind="ExternalInput")
    lnpack = nc.dram_tensor("lnpack", [L, 7 * D], bf16, kind="ExternalInput")
    de_t = nc.dram_tensor("de_t", [L, DH, C], bf16, kind="ExternalInput")
    de_rt = nc.dram_tensor("de_rt", [L, DH, C], bf16, kind="ExternalInput")
    ident_in = nc.dram_tensor("ident_in", [128, 128], bf16,
                              kind="ExternalInput")
    identr_in = nc.dram_tensor("identr_in", [128, 128], f32,
                               kind="ExternalInput")
    ones_col_in = nc.dram_tensor("ones_col_in", [128, 1], bf16,
                                 kind="ExternalInput")
    zsel_in = nc.dram_tensor("zsel_in", [128, 4], bf16,
                              kind="ExternalInput")
    bsel_in = nc.dram_tensor("bsel_in", [2, 128], f32,
                             kind="ExternalInput")
    y = nc.dram_tensor("y", [S, D], f32, kind="ExternalOutput")

    with TileContext(nc) as tc:
        with tc.tile_pool(name="resid", bufs=1) as p_res, \
             tc.tile_pool(name="fm", bufs=1) as p_fm, \
             tc.tile_pool(name="attn", bufs=1) as p_at, \
             tc.tile_pool(name="wpool", bufs=1) as p_w, \
             tc.tile_pool(name="cpool", bufs=1) as p_c, \
             tc.tile_pool(name="spool", bufs=2) as p_s, \
             tc.tile_pool(name="psum", bufs=1, space="PSUM") as p_ps:

            def acc_tile(i):
                return p_ps.tile([128, 512], f32, tag=f"acc{i}",
                                 name=f"acc{i}")

            def bnd_tile():
                return p_ps.tile([128, 1024], f32, tag="bnd", bufs=2,
                                 name="bnd")

            # ---- constants ----
            ident = p_c.tile([128, 128], bf16, tag="ident", name="ident")
            nc.sync.dma_start(ident[:], ident_in[:])
            ident_r = p_c.tile([128, 128], f32r, tag="identr", name="identr")
            nc.sync.dma_start(ident_r[:], identr_in[:].bitcast(f32r))
            ones_col = p_c.tile([128, 1], bf16, tag="onesc", name="ones_col")
            nc.sync.dma_start(ones_col[:], ones_col_in[:])
            zsel = p_c.tile([128, 4], bf16, tag="zsel", name="zsel")
            nc.sync.dma_start(zsel[:], zsel_in[:])
            bsel = p_c.tile([2, 128], f32r, tag="bsel", name="bsel")
            nc.sync.dma_start(bsel[:], bsel_in[:].bitcast(f32r))
            eps_c = p_c.tile([128, 1], f32, tag="eps", name="eps_c")
            nc.vector.memset(eps_c[:], EPS)
            invd_c = p_c.tile([128, 1], f32, tag="invd", name="invd_c")
            nc.vector.memset(invd_c[:], 1.0 / D)
            masks = []
            for t in range(NT):
                mt = p_c.tile([128, 1], f32, tag=f"mask{t}", name=f"mask{t}")
                nc.sync.dma_start(mt[:], mask_col[t * 128:(t + 1) * 128, :])
                masks.append(mt)

            # ---- LayerNorm: x_t f32 [128,D] -> xn (bf16, normalized) and
            # h_out = xn*g + b (f32 residual). g/b applied only to h_out.
            def layernorm(x_t, xn_out, h_out, g_ap, b_ap):
                sum_ = p_s.tile([128, 1], f32, tag="sum", name="sum")
                nc.vector.tensor_reduce(out=sum_[:], in_=x_t[:],
                                        axis=mybir.AxisListType.X, op=ALU.add)
                sq = p_s.tile([128, D], f32, tag="hp", name="sq")
                ssq = p_s.tile([128, 1], f32, tag="ssq", name="ssq")
                nc.scalar.activation(sq[:], x_t[:], AF.Square,
                                     accum_out=ssq[:])
                mu = p_s.tile([128, 1], f32, tag="mu", name="mu")
                nc.scalar.mul(mu[:], sum_[:], 1.0 / D)
                s2 = p_s.tile([128, 1], f32, tag="s2", name="s2")
                nc.vector.tensor_mul(s2[:], mu[:], mu[:])
                var = p_s.tile([128, 1], f32, tag="var", name="var")
                nc.vector.scalar_tensor_tensor(
                    out=var[:], in0=ssq[:], scalar=invd_c[:], in1=s2[:],
                    op0=ALU.mult, op1=ALU.subtract)
                lnv = p_s.tile([128, 1], f32, tag="std", name="lnv")
                nc.scalar.activation(lnv[:], var[:], AF.Ln, bias=eps_c[:])
                rstd = p_s.tile([128, 1], f32, tag="rstd", name="rstd")
                nc.scalar.activation(rstd[:], lnv[:], AF.Exp, scale=-0.5)
                nc.vector.scalar_tensor_tensor(
                    out=xn_out[:], in0=x_t[:], scalar=mu[:],
                    in1=rstd[:].to_broadcast((128, D)),
                    op0=ALU.subtract, op1=ALU.mult)
                tmp = p_s.tile([128, D], f32, tag="hp", name="lntmp")
                nc.vector.tensor_mul(tmp[:], xn_out[:], g_ap)
                nc.vector.tensor_add(h_out[:], tmp[:], b_ap)

            # ---- embedding ----
            xT_sb = p_c.tile([F, S], bf16, tag="xT", name="xT_sb")
            nc.sync.dma_start(xT_sb[:], xT[:])
            inw_sb = p_c.tile([F, D], bf16, tag="inw", name="inw_sb")
            nc.sync.dma_start(inw_sb[:], inw[:])
            ttib_bc = p_c.tile([128, D], bf16, tag="ttib", name="ttib_bc")
            nc.sync.dma_start(ttib_bc[:], bass.AP(
                tensor=ttib, offset=0, ap=[[0, 128], [1, D]]))
            embgb_bc = p_c.tile([128, 7 * D], bf16, tag="lnp",
                                name="embgb_bc")
            nc.sync.dma_start(embgb_bc[:, 0:2 * D], bass.AP(
                tensor=embgb, offset=0, ap=[[0, 128], [1, 2 * D]]))

            h, xn = [], []
            for t in range(NT):
                pe0 = acc_tile(t % 4)
                nc.tensor.matmul(pe0[:, 0:512],
                                 xT_sb[:, t * 128:(t + 1) * 128],
                                 inw_sb[:, 0:512], start=True, stop=True)
                pe1 = bnd_tile()
                nc.tensor.matmul(pe1[:, 0:256],
                                 xT_sb[:, t * 128:(t + 1) * 128],
                                 inw_sb[:, 512:768], start=True, stop=True)
                he = p_s.tile([128, D], f32, tag="hp", name="he")
                nc.vector.tensor_add(he[:, 0:512], pe0[:, 0:512],
                                     ttib_bc[:, 0:512])
                nc.vector.tensor_add(he[:, 512:768], pe1[:, 0:256],
                                     ttib_bc[:, 512:768])
                xnt = p_res.tile([128, D], f32r, tag=f"xn{t}", name=f"xn{t}")
                ht = p_res.tile([128, D], f32, tag=f"h{t}", name=f"h{t}")
                layernorm(he, xnt, ht, embgb_bc[:, 0:D], embgb_bc[:, D:2 * D])
                h.append(ht)
                xn.append(xnt)

            # ================= layers =================
            for l in range(L):
                # ---- weight / table / bias loads ----
                wq_sb = p_w.tile([128, KD * KD * 128], bf16, tag="wq",
                                 name="wq_sb")
                nc.gpsimd.dma_start(wq_sb[:], wq_p[l])
                wk_sb = p_w.tile([128, KD * KD * 128], bf16, tag="wk",
                                 name="wk_sb")
                nc.gpsimd.dma_start(wk_sb[:], wk_p[l])
                wv_sb = p_w.tile([128, KD * D], bf16, tag="wv", name="wv_sb")
                nc.gpsimd.dma_start(wv_sb[:], wv_p[l])
                wo_sb = p_w.tile([128, KD * D], bf16, tag="wo", name="wo_sb")
                nc.gpsimd.dma_start(wo_sb[:], wo_p[l])
                det_sb = p_c.tile([128, C + 1], bf16, tag="det",
                                  name="det_sb")
                nc.gpsimd.dma_start(det_sb[0:DH, 0:C], de_t[l])
                nc.gpsimd.dma_start(det_sb[DH:128, 0:C], de_t[l])
                dert_sb = p_c.tile([128, C + 1], bf16, tag="dert",
                                   name="dert_sb")
                nc.gpsimd.dma_start(dert_sb[0:DH, 0:C], de_rt[l])
                nc.gpsimd.dma_start(dert_sb[DH:128, 0:C], de_rt[l])
                lnp = p_c.tile([128, 7 * D], bf16, tag="lnp", name="lnp")
                nc.sync.dma_start(lnp[:], bass.AP(
                    tensor=lnpack, offset=l * 7 * D,
                    ap=[[0, 128], [1, 7 * D]]))
                bq_sb = p_c.tile([128, KD], f32, tag="bqc", name="bq_sb")
                nc.sync.dma_start(bq_sb[:], bass.AP(
                    tensor=bqc, offset=l * KD * 128,
                    ap=[[1, 128], [128, KD]]))
                bk_sb = p_c.tile([128, KD], f32, tag="bkc", name="bk_sb")
                nc.sync.dma_start(bk_sb[:], bass.AP(
                    tensor=bkc, offset=l * KD * 128,
                    ap=[[1, 128], [128, KD]]))
                b1_sb = p_c.tile([128, KI], f32, tag="b1c", name="b1_sb")
                nc.sync.dma_start(b1_sb[:], bass.AP(
                    tensor=b1c, offset=l * KI * 128,
                    ap=[[1, 128], [128, KI]]))
                bv_bc = lnp[:, 0 * D:1 * D]
                bo_bc = lnp[:, 1 * D:2 * D]
                b2_bc = lnp[:, 2 * D:3 * D]
                g1_bc = lnp[:, 3 * D:4 * D]
                be1_bc = lnp[:, 4 * D:5 * D]
                g2_bc = lnp[:, 5 * D:6 * D]
                be2_bc = lnp[:, 6 * D:7 * D]

                # ---- phase A: h_T feature-major via PE transposes of xn ----
                h_T = []
                for k in range(KD):
                    tp = acc_tile(k % 4).bitcast(f32r)
                    for t in range(NT):
                        nc.tensor.matmul(
                            tp[:, t * 128:(t + 1) * 128],
                            xn[t][:, k * 128:(k + 1) * 128], ident_r[:],
                            is_transpose=True, start=True, stop=True)
                    hTk = p_fm.tile([128, S], bf16, tag=f"hT{k}",
                                    name=f"hT{k}")
                    nc.scalar.activation(hTk[:], tp[:], AF.Copy)
                    h_T.append(hTk)

                # ---- phase B: Q^T, K^T feature-major ----
                q_T, k_T = [], []
                for e in range(KD):
                    psq = acc_tile(e % 2)
                    psk = acc_tile(2 + e % 2)
                    for k in range(KD):
                        co = (e * KD + k) * 128
                        nc.tensor.matmul(psq[:], wq_sb[:, co:co + 128],
                                         h_T[k][:],
                                         start=(k == 0), stop=(k == KD - 1))
                        nc.tensor.matmul(psk[:], wk_sb[:, co:co + 128],
                                         h_T[k][:],
                                         start=(k == 0), stop=(k == KD - 1))
                    qT = p_fm.tile([128, S], bf16, tag=f"qT{e}",
                                   name=f"qT{e}")
                    nc.scalar.activation(qT[:], psq[:], AF.Identity,
                                         bias=bq_sb[:, e:e + 1])
                    kT = p_fm.tile([128, S], bf16, tag=f"kT{e}",
                                   name=f"kT{e}")
                    nc.scalar.activation(kT[:], psk[:], AF.Identity,
                                         bias=bk_sb[:, e:e + 1])
                    q_T.append(qT)
                    k_T.append(kT)

                # ---- V token-major ----
                V = []
                for t in range(NT):
                    V.append(p_fm.tile([128, D], bf16, tag=f"V{t}",
                                       name=f"V{t}"))
                for half in range(2):
                    ts = (2 * half, 2 * half + 1)
                    pss = {}
                    for ti, t in enumerate(ts):
                        pss[(t, 0)] = acc_tile(2 * ti)
                        pss[(t, 1)] = acc_tile(2 * ti + 1)
                    for k in range(KD):
                        for t in ts:
                            nc.tensor.matmul(
                                pss[(t, 0)][:, 0:384],
                                h_T[k][:, t * 128:(t + 1) * 128],
                                wv_sb[:, k * D:k * D + 384],
                                start=(k == 0), stop=(k == KD - 1))
                            nc.tensor.matmul(
                                pss[(t, 1)][:, 0:384],
                                h_T[k][:, t * 128:(t + 1) * 128],
                                wv_sb[:, k * D + 384:k * D + 768],
                                start=(k == 0), stop=(k == KD - 1))
                    for t in ts:
                        nc.vector.tensor_add(V[t][:, 0:384],
                                             pss[(t, 0)][:, 0:384],
                                             bv_bc[:, 0:384])
                        nc.vector.tensor_add(V[t][:, 384:768],
                                             pss[(t, 1)][:, 0:384],
                                             bv_bc[:, 384:768])

                # ---- phase C: attention, one head pair per etile ----
                # C1: table bands + skew DMAs for both heads (q/k bands
                # interleaved so ACT and DVE copies run in parallel).
                # C2: scores/softmax/AV per head; skew latency of head r
                # hides under C1 of head r+1 / C2 of head r-1.
                ctx_T = []
                for e in range(KD):
                    skq, skk = [], []
                    for r in range(2):
                        qh = q_T[e][64 * r:64 * r + 64, :]
                        kh = k_T[e][64 * r:64 * r + 64, :]
                        dlo, dhi = 64 * r, 64 * r + 64
                        qba = p_at.tile([128, 4 * BAND], f32r, tag="qba",
                                        name="qba")
                        kba = p_at.tile([128, 4 * BAND], bf16, tag="kba",
                                        name="kba")
                        for qt in range(NT):
                            bs = 384 - 128 * qt
                            bnd = bnd_tile()
                            nc.tensor.matmul(
                                bnd[:, 0:512],
                                qh[:, qt * 128:(qt + 1) * 128],
                                dert_sb[dlo:dhi, bs:bs + 512],
                                start=True, stop=True)
                            nc.tensor.matmul(
                                bnd[:, 512:640],
                                qh[:, qt * 128:(qt + 1) * 128],
                                dert_sb[dlo:dhi, bs + 512:bs + 640],
                                start=True, stop=True)
                            nc.scalar.activation(
                                qba[:, qt * BAND:qt * BAND + 640],
                                bnd[:, 0:640], AF.Copy)
                            bnd = bnd_tile()
                            nc.tensor.matmul(
                                bnd[:, 0:512],
                                kh[:, qt * 128:(qt + 1) * 128],
                                det_sb[dlo:dhi, bs:bs + 512],
                                start=True, stop=True)
                            nc.tensor.matmul(
                                bnd[:, 512:640],
                                kh[:, qt * 128:(qt + 1) * 128],
                                det_sb[dlo:dhi, bs + 512:bs + 640],
                                start=True, stop=True)
                            nc.vector.tensor_copy(
                                kba[:, qt * BAND:qt * BAND + 640],
                                bnd[:, 0:640])
                        s2q = p_at.tile([128, 4 * S], f32r, tag="s2q",
                                        name="s2q")
                        nc.sync.dma_start(s2q[:], bass.AP(
                            tensor=qba.tensor, offset=qba.offset + 127,
                            ap=[[4 * BAND - 1, 128], [BAND, 4], [1, S]]))
                        s3t = p_at.tile([128, 4 * S], bf16, tag="s3t",
                                        name="s3t")
                        nc.sync.dma_start(s3t[:], bass.AP(
                            tensor=kba.tensor, offset=kba.offset + 127,
                            ap=[[4 * BAND - 1, 128], [BAND, 4], [1, S]]))
                        skq.append(s2q)
                        skk.append(s3t)

                    psc = None
                    z01 = None
                    for r in range(2):
                        qh = q_T[e][64 * r:64 * r + 64, :]
                        kh = k_T[e][64 * r:64 * r + 64, :]
                        st = [acc_tile(i) for i in range(4)]
                        for kt in range(4):
                            nc.tensor.matmul(
                                st[kt][:], kh[:, kt * 128:(kt + 1) * 128],
                                qh[:], start=True, stop=False)
                        # transpose-accumulate q-side into S^T
                        for qt in range(NT):
                            for kt in range(4):
                                nc.tensor.matmul(
                                    st[kt][:, qt * 128:(qt + 1) * 128]
                                    .bitcast(f32r),
                                    skq[r][:, qt * S + kt * 128:
                                           qt * S + (kt + 1) * 128],
                                    ident_r[:], is_transpose=True,
                                    start=False, stop=(qt == NT - 1))
                        # k-side add + exp
                        pTs = []
                        for kt in range(4):
                            nc.vector.tensor_add(
                                st[kt][:], st[kt][:],
                                skk[r][:, kt * S:(kt + 1) * S])
                            pT = p_at.tile([128, S], bf16, tag="pT",
                                           bufs=4, name="pT")
                            nc.scalar.activation(pT[:], st[kt][:], AF.Exp,
                                                 bias=masks[kt][:],
                                                 scale=float(SCALE))
                            pTs.append(pT)
                        # Z rows for both heads accumulate into z01[0:2]
                        if r == 0:
                            z01 = bnd_tile()
                        for kt in range(4):
                            nc.tensor.matmul(
                                z01[0:2, 0:512], zsel[:, 2 * r:2 * r + 2],
                                pTs[kt][:],
                                start=(r == 0 and kt == 0),
                                stop=(r == 1 and kt == 3))
                        # AV for this head into shared psc
                        if r == 0:
                            psc = bnd_tile()
                        hh = 2 * e + r
                        for kt in range(4):
                            nc.tensor.matmul(
                                psc[64 * r:64 * r + 64, 0:512],
                                V[kt][:, hh * 64:hh * 64 + 64],
                                pTs[kt][:],
                                start=(kt == 0), stop=(kt == 3))
                    lnz = p_at.tile([2, S], f32, tag="lnz", name="lnz")
                    nc.scalar.activation(lnz[:], z01[0:2, 0:512], AF.Ln)
                    rz01 = p_at.tile([2, S], f32r, tag="z_sb", name="rz01")
                    nc.scalar.activation(rz01[:], lnz[:], AF.Exp, scale=-1.0)
                    zbc = acc_tile(0)
                    nc.tensor.matmul(zbc[:, :], bsel[:], rz01[:],
                                     start=True, stop=True)
                    rzbs = p_at.tile([128, S], bf16, tag="rzs", name="rzbs")
                    nc.scalar.activation(rzbs[:], zbc[:], AF.Copy)
                    cT = p_fm.tile([128, S], bf16, tag=f"qT{e}",
                                   name=f"cT{e}")
                    nc.vector.tensor_mul(cT[:], psc[0:128, 0:512], rzbs[:])
                    ctx_T.append(cT)

                # ---- phase D: O-proj + residual + LN1 ----
                h1, xn1 = [], []
                for half in range(2):
                    ts = (2 * half, 2 * half + 1)
                    pss = {}
                    for ti, t in enumerate(ts):
                        pss[(t, 0)] = acc_tile(2 * ti)
                        pss[(t, 1)] = acc_tile(2 * ti + 1)
                    for k in range(KD):
                        for t in ts:
                            nc.tensor.matmul(
                                pss[(t, 0)][:, 0:384],
                                ctx_T[k][:, t * 128:(t + 1) * 128],
                                wo_sb[:, k * D:k * D + 384],
                                start=(k == 0), stop=(k == KD - 1))
                            nc.tensor.matmul(
                                pss[(t, 1)][:, 0:384],
                                ctx_T[k][:, t * 128:(t + 1) * 128],
                                wo_sb[:, k * D + 384:k * D + 768],
                                start=(k == 0), stop=(k == KD - 1))
                    for t in ts:
                        hp = p_s.tile([128, D], f32, tag="hp", name="hp")
                        nc.vector.tensor_add(hp[:, 0:384],
                                             pss[(t, 0)][:, 0:384],
                                             h[t][:, 0:384])
                        nc.vector.tensor_add(hp[:, 384:768],
                                             pss[(t, 1)][:, 0:384],
                                             h[t][:, 384:768])
                        nc.vector.tensor_add(hp[:], hp[:], bo_bc[:])
                        xnt = p_res.tile([128, D], f32r, tag=f"xn{t}",
                                         name=f"xn1_{t}")
                        h1t = p_res.tile([128, D], f32, tag=f"h1_{t}",
                                         name=f"h1_{t}")
                        layernorm(hp, xnt, h1t, g1_bc, be1_bc)
                        xn1.append(xnt)
                        h1.append(h1t)

                # ---- phase E: FFN ----
                h1_T = []
                for k in range(KD):
                    tp = acc_tile(k % 4).bitcast(f32r)
                    for t in range(NT):
                        nc.tensor.matmul(
                            tp[:, t * 128:(t + 1) * 128],
                            xn1[t][:, k * 128:(k + 1) * 128], ident_r[:],
                            is_transpose=True, start=True, stop=True)
                    hTk = p_fm.tile([128, S], bf16, tag=f"hT{k}",
                                    name=f"h1T{k}")
                    nc.scalar.activation(hTk[:], tp[:], AF.Copy)
                    h1_T.append(hTk)

                for blk in range(4):
                    w1c = p_w.tile([128, 6 * KD * 128], bf16, tag="w1c",
                                   bufs=2, name="w1c")
                    nc.gpsimd.dma_start(
                        w1c[:],
                        w1_p[l, :, blk * 6 * KD * 128:
                             (blk + 1) * 6 * KD * 128])
                    w2c = p_w.tile([128, 6 * D], bf16, tag="w2c",
                                   bufs=2, name="w2c")
                    nc.gpsimd.dma_start(
                        w2c[:], w2_p[l, :, blk * 6 * D:(blk + 1) * 6 * D])
                    g_T = []
                    for j in range(6):
                        i = blk * 6 + j
                        ps = acc_tile(j % 2)
                        for k in range(KD):
                            co = (j * KD + k) * 128
                            nc.tensor.matmul(
                                ps[:], w1c[:, co:co + 128], h1_T[k][:],
                                start=(k == 0), stop=(k == KD - 1))
                        gt = p_fm.tile([128, S], bf16, tag=f"gT{j}",
                                       name=f"gT{j}")
                        nc.scalar.activation(gt[:], ps[:], AF.Gelu,
                                             bias=b1_sb[:, i:i + 1])
                        g_T.append(gt)
                    for half in range(2):
                        ts = (2 * half, 2 * half + 1)
                        pss = {}
                        for ti, t in enumerate(ts):
                            pss[(t, 0)] = acc_tile(2 + ti)
                            pss[(t, 1)] = bnd_tile()
                        for j in range(6):
                            for t in ts:
                                nc.tensor.matmul(
                                    pss[(t, 0)][:, 0:384],
                                    g_T[j][:, t * 128:(t + 1) * 128],
                                    w2c[:, j * D:j * D + 384],
                                    start=(j == 0), stop=(j == 5))
                                nc.tensor.matmul(
                                    pss[(t, 1)][:, 0:384],
                                    g_T[j][:, t * 128:(t + 1) * 128],
                                    w2c[:, j * D + 384:j * D + 768],
                                    start=(j == 0), stop=(j == 5))
                        for t in ts:
                            nc.vector.tensor_add(h1[t][:, 0:384],
                                                 h1[t][:, 0:384],
                                                 pss[(t, 0)][:, 0:384])
                            nc.vector.tensor_add(h1[t][:, 384:768],
                                                 h1[t][:, 384:768],
                                                 pss[(t, 1)][:, 0:384])

                new_h, new_xn = [], []
                for t in range(NT):
                    nc.vector.tensor_add(h1[t][:], h1[t][:], b2_bc[:])
                    xnt = p_res.tile([128, D], f32r, tag=f"xn{t}",
                                     name=f"nxn{t}")
                    ht = p_res.tile([128, D], f32, tag=f"h{t}",
                                    name=f"nh{t}")
                    layernorm(h1[t], xnt, ht, g2_bc, be2_bc)
                    new_h.append(ht)
                    new_xn.append(xnt)
                h, xn = new_h, new_xn

            for t in range(NT):
                nc.sync.dma_start(y[t * 128:(t + 1) * 128, :], h[t][:])

    return nc


def _prep_inputs(inputs):
    fp = np.float32
    b16 = ml_dtypes.bfloat16
    ii = np.ascontiguousarray(inputs["input_ids"], fp)
    am = np.ascontiguousarray(inputs["attn_mask"], fp)
    de = np.asarray(inputs["dist_emb"], fp)
    wq = np.asarray(inputs["wq"], fp)
    wk = np.asarray(inputs["wk"], fp)
    wv = np.asarray(inputs["wv"], fp)
    wo = np.asarray(inputs["wo"], fp)
    w1 = np.asarray(inputs["w1"], fp)
    w2 = np.asarray(inputs["w2"], fp)
    bq = np.asarray(inputs["bq"], fp)
    bk = np.asarray(inputs["bk"], fp)
    bv = np.asarray(inputs["bv"], fp)
    bo = np.asarray(inputs["bo"], fp)
    b1 = np.asarray(inputs["b1"], fp)
    b2 = np.asarray(inputs["b2"], fp)
    g1 = np.asarray(inputs["ln1_g"], fp)
    be1 = np.asarray(inputs["ln1_b"], fp)
    g2 = np.asarray(inputs["ln2_g"], fp)
    be2 = np.asarray(inputs["ln2_b"], fp)
    emb_g = np.asarray(inputs["emb_ln_g"], fp)
    emb_b = np.asarray(inputs["emb_ln_b"], fp)

    # fold the preceding LN affine into Wq/Wk/Wv; fold ln1 into W1
    gprev = np.stack([emb_g] + [g2[ll] for ll in range(L - 1)])
    bprev = np.stack([emb_b] + [be2[ll] for ll in range(L - 1)])
    wqf = gprev[:, :, None] * wq
    wkf = gprev[:, :, None] * wk
    wvf = gprev[:, :, None] * wv
    bqf = bq + np.einsum('ld,lde->le', bprev, wq)
    bkf = bk + np.einsum('ld,lde->le', bprev, wk)
    bvf = bv + np.einsum('ld,lde->le', bprev, wv)
    w1f = g1[:, :, None] * w1
    b1f = b1 + np.einsum('ld,ldi->li', be1, w1)

    def cb(x):
        return np.ascontiguousarray(x.astype(b16))

    shared = dict(
        inw=cb(np.asarray(inputs["in_w"], fp)),
        ttib=cb(np.asarray(inputs["in_b"] + inputs["tte"], fp)),
        embgb=cb(np.concatenate([emb_g, emb_b])),
        wq_p=cb(wqf.reshape(L, KD, 128, KD, 128)
                .transpose(0, 2, 3, 1, 4).reshape(L, 128, KD * KD * 128)),
        wk_p=cb(wkf.reshape(L, KD, 128, KD, 128)
                .transpose(0, 2, 3, 1, 4).reshape(L, 128, KD * KD * 128)),
        wv_p=cb(wvf.reshape(L, KD, 128, D)
                .transpose(0, 2, 1, 3).reshape(L, 128, KD * D)),
        wo_p=cb(wo.reshape(L, KD, 128, D)
                .transpose(0, 2, 1, 3).reshape(L, 128, KD * D)),
        w1_p=cb(w1f.reshape(L, KD, 128, KI, 128)
                .transpose(0, 2, 3, 1, 4).reshape(L, 128, KI * KD * 128)),
        w2_p=cb(w2.reshape(L, KI, 128, D)
                .transpose(0, 2, 1, 3).reshape(L, 128, KI * D)),
        bqc=np.ascontiguousarray(bqf.reshape(L, KD, 128), fp),
        bkc=np.ascontiguousarray(bkf.reshape(L, KD, 128), fp),
        b1c=np.ascontiguousarray(b1f.reshape(L, KI, 128), fp),
        lnpack=cb(np.stack([bvf, bo, b2, g1, be1, g2, be2], axis=1)
                  .reshape(L, 7 * D)),
        de_t=cb(de.transpose(0, 2, 1)),
        de_rt=cb(de[:, ::-1, :].transpose(0, 2, 1)),
        ident_in=cb(np.eye(128, dtype=fp)),
        identr_in=np.eye(128, dtype=fp),
        ones_col_in=cb(np.ones((128, 1), fp)),
        zsel_in=cb(np.stack([np.ones(128), np.zeros(128),
                             np.zeros(128), np.ones(128)], axis=1)),
        bsel_in=np.ascontiguousarray(np.stack([
            np.concatenate([np.ones(64), np.zeros(64)]),
            np.concatenate([np.zeros(64), np.ones(64)])]), fp),
    )
    in_maps = []
    for c in range(B):
        m = dict(shared)
        m["xT"] = cb(ii[c].T)
        m["mask_col"] = np.ascontiguousarray(
            ((1.0 - am[c]) * -1e9)[:, None], fp)
        in_maps.append(m)
    return in_maps


def kernel(trace=False, **inputs):
    if "nc" not in _CACHED:
        _CACHED["nc"] = build_module()
    nc = _CACHED["nc"]
    in_maps = _prep_inputs(inputs)
    res = bass_utils.run_bass_kernel_spmd(
        nc, in_maps, core_ids=list(range(B)), trace=trace)
    out = np.stack([np.asarray(res.results[c]["y"], np.float32)
                    for c in range(B)])
    if trace:
        kernel.last_exec_time_ns = res.exec_time_ns
        kernel.last_results = res
    return out


# revision 13
# speedup vs baseline: 1.0515x; 1.0096x over previous
"""ExpressionBert Trainium2 kernel (v2).

Data-parallel over batch: 8 batch elements -> 8 NeuronCores, no collectives.
Per core: 512 tokens through 6 post-LN transformer layers with
relative_key_query attention.

v2 changes vs baseline:
  - bf16 matmul operands everywhere (weights host-converted, activations
    evicted from PSUM as bf16).  PSUM accumulation and the residual
    stream stay f32.
  - LN affine (g,b) of the preceding LN is folded into Wq/Wk/Wv/W1 on the
    host, so the PE-critical path needs only the normalized xn, not the
    affine output.  The affine is applied off-critical-path for the
    residual stream only.
  - Host-permuted weight layouts -> one contiguous DMA per weight matrix
    per layer (W1/W2 in 4 chunks), issued on the GpSimd SWDGE queue so
    they don't head-of-line block the skew DMAs on the sync queue.
  - Rel-position tables: 4 bands are copied into one flat SBUF tile and
    skewed with a single 3D-AP DMA per (head, side) - 24 skew DMAs/layer
    instead of 96.  k-side bands/skews in bf16 (DVE-copied), q-side stays
    f32 because its skew feeds PE transpose-accumulation into the score
    PSUM.
  - 1/Z partition-broadcast via a PE matmul (ones outer product) instead
    of a DRAM round trip.
"""

import numpy as np
import ml_dtypes

import bass_rust
import concourse.bass as bass
import concourse.mybir as mybir
from concourse import bass_utils
from concourse import tile as tile_mod

f32 = mybir.dt.float32
f32r = mybir.dt.float32r
bf16 = mybir.dt.bfloat16
AF = mybir.ActivationFunctionType
ALU = mybir.AluOpType

# ---- walrus workaround: only ONE sem wait per instruction is supported ----


def _split_multi_waits(nc):
    for f in nc.m.functions:
        for bb in f.blocks:
            new = []
            dirty = False
            for ins in bb.instructions:
                si = ins.sync_info
                if si is not None and len(si.on_wait) > 1:
                    waits = list(si.on_wait)
                    for w in waits[:-1]:
                        nop = mybir.InstNoOp(
                            name=f"waitnop-{nc.next_id()}", ins=[], outs=[])
                        nop.engine = ins.engine
                        nop.sync_info = bass_rust.SyncInfo(
                            on_wait=[w], on_update=[])
                        new.append(nop)
                    ins.sync_info = bass_rust.SyncInfo(
                        on_wait=[waits[-1]], on_update=list(si.on_update))
                    dirty = True
                new.append(ins)
            if dirty:
                bb.instructions = new


class TileContext(tile_mod.TileContext):
    def __exit__(self, exc_type, exc_value, traceback):
        r = super().__exit__(exc_type, exc_value, traceback)
        if exc_type is None:
            _split_multi_waits(self.nc)
        return r


# ---- model dims ----
B, S, F, D, L, H, I = 8, 512, 5, 768, 6, 12, 3072
DH = 64              # head dim
KD = 6               # D / 128
KI = 24              # I / 128
NT = 4               # S / 128
C = 1023             # 2M-1 relative positions
BAND = 640           # per-chunk table band width (639 used + 1 pad)
SCALE = 1.0 / np.sqrt(DH)
EPS = 1e-12

_CACHED = {}


def build_module():
    nc = bass.Bass()

    # ---------------- DRAM I/O ----------------
    xT = nc.dram_tensor("xT", [F, S], bf16, kind="ExternalInput")
    mask_col = nc.dram_tensor("mask_col", [S, 1], f32, kind="ExternalInput")
    inw = nc.dram_tensor("inw", [F, D], bf16, kind="ExternalInput")
    ttib = nc.dram_tensor("ttib", [D], bf16, kind="ExternalInput")
    embgb = nc.dram_tensor("embgb", [2 * D], bf16, kind="ExternalInput")
    wq_p = nc.dram_tensor("wq_p", [L, 128, KD * KD * 128], bf16,
                          kind="ExternalInput")
    wk_p = nc.dram_tensor("wk_p", [L, 128, KD * KD * 128], bf16,
                          kind="ExternalInput")
    wv_p = nc.dram_tensor("wv_p", [L, 128, KD * D], bf16,
                          kind="ExternalInput")
    wo_p = nc.dram_tensor("wo_p", [L, 128, KD * D], bf16,
                          kind="ExternalInput")
    w1_p = nc.dram_tensor("w1_p", [L, 128, KI * KD * 128], bf16,
                          kind="ExternalInput")
    w2_p = nc.dram_tensor("w2_p", [L, 128, KI * D], bf16,
                          kind="ExternalInput")
    bqc = nc.dram_tensor("bqc", [L, KD, 128], f32, kind="ExternalInput")
    bkc = nc.dram_tensor("bkc", [L, KD, 128], f32, kind="ExternalInput")
    b1c = nc.dram_tensor("b1c", [L, KI, 128], f32, kind="ExternalInput")
    lnpack = nc.dram_tensor("lnpack", [L, 7 * D], bf16, kind="ExternalInput")
    de_t = nc.dram_tensor("de_t", [L, DH, C], bf16, kind="ExternalInput")
    de_rt = nc.dram_tensor("de_rt", [L, DH, C], bf16, kind="ExternalInput")
    ident_in = nc.dram_tensor("ident_in", [128, 128], bf16,
                              kind="ExternalInput")
    identr_in = nc.dram_tensor("identr_in", [128, 128], f32,
                               kind="ExternalInput")
    ones_col_in = nc.dram_tensor("ones_col_in", [128, 1], bf16,
                                 kind="ExternalInput")
    zsel_in = nc.dram_tensor("zsel_in", [128, 4], bf16,
                              kind="ExternalInput")
    bsel_in = nc.dram_tensor("bsel_in", [2, 128], f32,
                             kind="ExternalInput")
    y = nc.dram_tensor("y", [S, D], f32, kind="ExternalOutput")

    with TileContext(nc) as tc:
        with tc.tile_pool(name="resid", bufs=1) as p_res, \
             tc.tile_pool(name="fm", bufs=1) as p_fm, \
             tc.tile_pool(name="attn", bufs=1) as p_at, \
             tc.tile_pool(name="wpool", bufs=1) as p_w, \
             tc.tile_pool(name="cpool", bufs=1) as p_c, \
             tc.tile_pool(name="spool", bufs=2) as p_s, \
             tc.tile_pool(name="psum", bufs=1, space="PSUM") as p_ps:

            def acc_tile(i):
                return p_ps.tile([128, 512], f32, tag=f"acc{i}",
                                 name=f"acc{i}")

            def bnd_tile():
                return p_ps.tile([128, 1024], f32, tag="bnd", bufs=2,
                                 name="bnd")

            # ---- constants ----
            ident = p_c.tile([128, 128], bf16, tag="ident", name="ident")
            nc.sync.dma_start(ident[:], ident_in[:])
            ident_r = p_c.tile([128, 128], f32r, tag="identr", name="identr")
            nc.sync.dma_start(ident_r[:], identr_in[:].bitcast(f32r))
            ones_col = p_c.tile([128, 1], bf16, tag="onesc", name="ones_col")
            nc.sync.dma_start(ones_col[:], ones_col_in[:])
            zsel = p_c.tile([128, 4], bf16, tag="zsel", name="zsel")
            nc.sync.dma_start(zsel[:], zsel_in[:])
            bsel = p_c.tile([2, 128], f32r, tag="bsel", name="bsel")
            nc.sync.dma_start(bsel[:], bsel_in[:].bitcast(f32r))
            eps_c = p_c.tile([128, 1], f32, tag="eps", name="eps_c")
            nc.vector.memset(eps_c[:], EPS)
            invd_c = p_c.tile([128, 1], f32, tag="invd", name="invd_c")
            nc.vector.memset(invd_c[:], 1.0 / D)
            masks = []
            for t in range(NT):
                mt = p_c.tile([128, 1], f32, tag=f"mask{t}", name=f"mask{t}")
                nc.sync.dma_start(mt[:], mask_col[t * 128:(t + 1) * 128, :])
                masks.append(mt)

            # ---- LayerNorm: x_t f32 [128,D] -> xn (bf16, normalized) and
            # h_out = xn*g + b (f32 residual). g/b applied only to h_out.
            def layernorm(x_t, xn_out, h_out, g_ap, b_ap):
                sum_ = p_s.tile([128, 1], f32, tag="sum", name="sum")
                nc.vector.tensor_reduce(out=sum_[:], in_=x_t[:],
                                        axis=mybir.AxisListType.X, op=ALU.add)
                sq = p_s.tile([128, D], f32, tag="hp", name="sq")
                ssq = p_s.tile([128, 1], f32, tag="ssq", name="ssq")
                nc.scalar.activation(sq[:], x_t[:], AF.Square,
                                     accum_out=ssq[:])
                mu = p_s.tile([128, 1], f32, tag="mu", name="mu")
                nc.scalar.mul(mu[:], sum_[:], 1.0 / D)
                s2 = p_s.tile([128, 1], f32, tag="s2", name="s2")
                nc.vector.tensor_mul(s2[:], mu[:], mu[:])
                var = p_s.tile([128, 1], f32, tag="var", name="var")
                nc.vector.scalar_tensor_tensor(
                    out=var[:], in0=ssq[:], scalar=invd_c[:], in1=s2[:],
                    op0=ALU.mult, op1=ALU.subtract)
                lnv = p_s.tile([128, 1], f32, tag="std", name="lnv")
                nc.scalar.activation(lnv[:], var[:], AF.Ln, bias=eps_c[:])
                rstd = p_s.tile([128, 1], f32, tag="rstd", name="rstd")
                nc.scalar.activation(rstd[:], lnv[:], AF.Exp, scale=-0.5)
                nc.vector.scalar_tensor_tensor(
                    out=xn_out[:], in0=x_t[:], scalar=mu[:],
                    in1=rstd[:].to_broadcast((128, D)),
                    op0=ALU.subtract, op1=ALU.mult)
                tmp = p_s.tile([128, D], f32, tag="hp", name="lntmp")
                nc.vector.tensor_mul(tmp[:], xn_out[:], g_ap)
                nc.vector.tensor_add(h_out[:], tmp[:], b_ap)

            # ---- embedding ----
            xT_sb = p_c.tile([F, S], bf16, tag="xT", name="xT_sb")
            nc.sync.dma_start(xT_sb[:], xT[:])
            inw_sb = p_c.tile([F, D], bf16, tag="inw", name="inw_sb")
            nc.sync.dma_start(inw_sb[:], inw[:])
            ttib_bc = p_c.tile([128, D], bf16, tag="ttib", name="ttib_bc")
            nc.sync.dma_start(ttib_bc[:], bass.AP(
                tensor=ttib, offset=0, ap=[[0, 128], [1, D]]))
            embgb_bc = p_c.tile([128, 7 * D], bf16, tag="lnp",
                                name="embgb_bc")
            nc.sync.dma_start(embgb_bc[:, 0:2 * D], bass.AP(
                tensor=embgb, offset=0, ap=[[0, 128], [1, 2 * D]]))

            h, xn = [], []
            for t in range(NT):
                pe0 = acc_tile(t % 4)
                nc.tensor.matmul(pe0[:, 0:512],
                                 xT_sb[:, t * 128:(t + 1) * 128],
                                 inw_sb[:, 0:512], start=True, stop=True)
                pe1 = bnd_tile()
                nc.tensor.matmul(pe1[:, 0:256],
                                 xT_sb[:, t * 128:(t + 1) * 128],
                                 inw_sb[:, 512:768], start=True, stop=True)
                he = p_s.tile([128, D], f32, tag="hp", name="he")
                nc.vector.tensor_add(he[:, 0:512], pe0[:, 0:512],
                                     ttib_bc[:, 0:512])
                nc.vector.tensor_add(he[:, 512:768], pe1[:, 0:256],
                                     ttib_bc[:, 512:768])
                xnt = p_res.tile([128, D], f32r, tag=f"xn{t}", name=f"xn{t}")
                ht = p_res.tile([128, D], f32, tag=f"h{t}", name=f"h{t}")
                layernorm(he, xnt, ht, embgb_bc[:, 0:D], embgb_bc[:, D:2 * D])
                h.append(ht)
                xn.append(xnt)

            # ================= layers =================
            for l in range(L):
                # ---- weight / table / bias loads ----
                wq_sb = p_w.tile([128, KD * KD * 128], bf16, tag="wq",
                                 name="wq_sb")
                nc.gpsimd.dma_start(wq_sb[:], wq_p[l])
                wk_sb = p_w.tile([128, KD * KD * 128], bf16, tag="wk",
                                 name="wk_sb")
                nc.gpsimd.dma_start(wk_sb[:], wk_p[l])
                wv_sb = p_w.tile([128, KD * D], bf16, tag="wv", name="wv_sb")
                nc.gpsimd.dma_start(wv_sb[:], wv_p[l])
                wo_sb = p_w.tile([128, KD * D], bf16, tag="wo", name="wo_sb")
                nc.gpsimd.dma_start(wo_sb[:], wo_p[l])
                det_sb = p_c.tile([128, C + 1], bf16, tag="det",
                                  name="det_sb")
                nc.gpsimd.dma_start(det_sb[0:DH, 0:C], de_t[l])
                nc.gpsimd.dma_start(det_sb[DH:128, 0:C], de_t[l])
                dert_sb = p_c.tile([128, C + 1], bf16, tag="dert",
                                   name="dert_sb")
                nc.gpsimd.dma_start(dert_sb[0:DH, 0:C], de_rt[l])
                nc.gpsimd.dma_start(dert_sb[DH:128, 0:C], de_rt[l])
                lnp = p_c.tile([128, 7 * D], bf16, tag="lnp", name="lnp")
                nc.sync.dma_start(lnp[:], bass.AP(
                    tensor=lnpack, offset=l * 7 * D,
                    ap=[[0, 128], [1, 7 * D]]))
                bq_sb = p_c.tile([128, KD], f32, tag="bqc", name="bq_sb")
                nc.sync.dma_start(bq_sb[:], bass.AP(
                    tensor=bqc, offset=l * KD * 128,
                    ap=[[1, 128], [128, KD]]))
                bk_sb = p_c.tile([128, KD], f32, tag="bkc", name="bk_sb")
                nc.sync.dma_start(bk_sb[:], bass.AP(
                    tensor=bkc, offset=l * KD * 128,
                    ap=[[1, 128], [128, KD]]))
                b1_sb = p_c.tile([128, KI], f32, tag="b1c", name="b1_sb")
                nc.sync.dma_start(b1_sb[:], bass.AP(
                    tensor=b1c, offset=l * KI * 128,
                    ap=[[1, 128], [128, KI]]))
                bv_bc = lnp[:, 0 * D:1 * D]
                bo_bc = lnp[:, 1 * D:2 * D]
                b2_bc = lnp[:, 2 * D:3 * D]
                g1_bc = lnp[:, 3 * D:4 * D]
                be1_bc = lnp[:, 4 * D:5 * D]
                g2_bc = lnp[:, 5 * D:6 * D]
                be2_bc = lnp[:, 6 * D:7 * D]

                # ---- phase A: h_T feature-major via PE transposes of xn ----
                h_T = []
                tpv = [acc_tile(i).bitcast(f32r) for i in range(4)]
                tpv.append(bnd_tile()[:, 0:512].bitcast(f32r))
                tpv.append(bnd_tile()[:, 0:512].bitcast(f32r))
                for t in range(NT):
                    for k in range(KD):
                        nc.tensor.matmul(
                            tpv[k][:, t * 128:(t + 1) * 128],
                            xn[t][:, k * 128:(k + 1) * 128], ident_r[:],
                            is_transpose=True, start=True, stop=True)
                for k in range(KD):
                    hTk = p_fm.tile([128, S], bf16, tag=f"hT{k}",
                                    name=f"hT{k}")
                    nc.scalar.activation(hTk[:], tpv[k][:], AF.Copy)
                    h_T.append(hTk)

                # ---- phase B: Q^T, K^T feature-major ----
                q_T, k_T = [], []
                for e in range(KD):
                    psq = acc_tile(e % 2)
                    psk = acc_tile(2 + e % 2)
                    for k in range(KD):
                        co = (e * KD + k) * 128
                        nc.tensor.matmul(psq[:], wq_sb[:, co:co + 128],
                                         h_T[k][:],
                                         start=(k == 0), stop=(k == KD - 1))
                        nc.tensor.matmul(psk[:], wk_sb[:, co:co + 128],
                                         h_T[k][:],
                                         start=(k == 0), stop=(k == KD - 1))
                    qT = p_fm.tile([128, S], bf16, tag=f"qT{e}",
                                   name=f"qT{e}")
                    nc.scalar.activation(qT[:], psq[:], AF.Identity,
                                         bias=bq_sb[:, e:e + 1])
                    kT = p_fm.tile([128, S], bf16, tag=f"kT{e}",
                                   name=f"kT{e}")
                    nc.scalar.activation(kT[:], psk[:], AF.Identity,
                                         bias=bk_sb[:, e:e + 1])
                    q_T.append(qT)
                    k_T.append(kT)

                # ---- V token-major ----
                V = []
                for t in range(NT):
                    V.append(p_fm.tile([128, D], bf16, tag=f"V{t}",
                                       name=f"V{t}"))
                for half in range(2):
                    ts = (2 * half, 2 * half + 1)
                    pss = {}
                    for ti, t in enumerate(ts):
                        pss[(t, 0)] = acc_tile(2 * ti)
                        pss[(t, 1)] = acc_tile(2 * ti + 1)
                    for k in range(KD):
                        for t in ts:
                            nc.tensor.matmul(
                                pss[(t, 0)][:, 0:384],
                                h_T[k][:, t * 128:(t + 1) * 128],
                                wv_sb[:, k * D:k * D + 384],
                                start=(k == 0), stop=(k == KD - 1))
                            nc.tensor.matmul(
                                pss[(t, 1)][:, 0:384],
                                h_T[k][:, t * 128:(t + 1) * 128],
                                wv_sb[:, k * D + 384:k * D + 768],
                                start=(k == 0), stop=(k == KD - 1))
                    for t in ts:
                        nc.vector.tensor_add(V[t][:, 0:384],
                                             pss[(t, 0)][:, 0:384],
                                             bv_bc[:, 0:384])
                        nc.vector.tensor_add(V[t][:, 384:768],
                                             pss[(t, 1)][:, 0:384],
                                             bv_bc[:, 384:768])

                # ---- phase C: attention, one head pair per etile ----
                # C1: table bands + skew DMAs for both heads (q/k bands
                # interleaved so ACT and DVE copies run in parallel).
                # C2: scores/softmax/AV per head; skew latency of head r
                # hides under C1 of head r+1 / C2 of head r-1.
                ctx_T = []
                for e in range(KD):
                    skq, skk = [], []
                    for r in range(2):
                        qh = q_T[e][64 * r:64 * r + 64, :]
                        kh = k_T[e][64 * r:64 * r + 64, :]
                        dlo, dhi = 64 * r, 64 * r + 64
                        qba = p_at.tile([128, 4 * BAND], f32r, tag="qba",
                                        name="qba")
                        kba = p_at.tile([128, 4 * BAND], bf16, tag="kba",
                                        name="kba")
                        for qt in range(NT):
                            bs = 384 - 128 * qt
                            bnd = bnd_tile()
                            nc.tensor.matmul(
                                bnd[:, 0:512],
                                qh[:, qt * 128:(qt + 1) * 128],
                                dert_sb[dlo:dhi, bs:bs + 512],
                                start=True, stop=True)
                            nc.tensor.matmul(
                                bnd[:, 512:640],
                                qh[:, qt * 128:(qt + 1) * 128],
                                dert_sb[dlo:dhi, bs + 512:bs + 640],
                                start=True, stop=True)
                            nc.scalar.activation(
                                qba[:, qt * BAND:qt * BAND + 640],
                                bnd[:, 0:640], AF.Copy)
                            bnd = bnd_tile()
                            nc.tensor.matmul(
                                bnd[:, 0:512],
                                kh[:, qt * 128:(qt + 1) * 128],
                                det_sb[dlo:dhi, bs:bs + 512],
                                start=True, stop=True)
                            nc.tensor.matmul(
                                bnd[:, 512:640],
                                kh[:, qt * 128:(qt + 1) * 128],
                                det_sb[dlo:dhi, bs + 512:bs + 640],
                                start=True, stop=True)
                            nc.vector.tensor_copy(
                                kba[:, qt * BAND:qt * BAND + 640],
                                bnd[:, 0:640])
                        s2q = p_at.tile([128, 4 * S], f32r, tag="s2q",
                                        bufs=2, name="s2q")
                        nc.sync.dma_start(s2q[:], bass.AP(
                            tensor=qba.tensor, offset=qba.offset + 127,
                            ap=[[4 * BAND - 1, 128], [BAND, 4], [1, S]]))
                        s3t = p_at.tile([128, 4 * S], bf16, tag="s3t",
                                        bufs=2, name="s3t")
                        nc.sync.dma_start(s3t[:], bass.AP(
                            tensor=kba.tensor, offset=kba.offset + 127,
                            ap=[[4 * BAND - 1, 128], [BAND, 4], [1, S]]))
                        skq.append(s2q)
                        skk.append(s3t)

                    psc = None
                    z01 = None
                    for r in range(2):
                        qh = q_T[e][64 * r:64 * r + 64, :]
                        kh = k_T[e][64 * r:64 * r + 64, :]
                        st = [acc_tile(i) for i in range(4)]
                        for kt in range(4):
                            nc.tensor.matmul(
                                st[kt][:], kh[:, kt * 128:(kt + 1) * 128],
                                qh[:], start=True, stop=False)
                        # transpose-accumulate q-side into S^T
                        for qt in range(NT):
                            for kt in range(4):
                                nc.tensor.matmul(
                                    st[kt][:, qt * 128:(qt + 1) * 128]
                                    .bitcast(f32r),
                                    skq[r][:, qt * S + kt * 128:
                                           qt * S + (kt + 1) * 128],
                                    ident_r[:], is_transpose=True,
                                    start=False, stop=(qt == NT - 1))
                        # k-side add + exp
                        pTs = []
                        for kt in range(4):
                            nc.vector.tensor_add(
                                st[kt][:], st[kt][:],
                                skk[r][:, kt * S:(kt + 1) * S])
                            pT = p_at.tile([128, S], bf16, tag="pT",
                                           bufs=4, name="pT")
                            nc.scalar.activation(pT[:], st[kt][:], AF.Exp,
                                                 bias=masks[kt][:],
                                                 scale=float(SCALE))
                            pTs.append(pT)
                        # Z rows for both heads accumulate into z01[0:2]
                        if r == 0:
                            z01 = bnd_tile()
                        for kt in range(4):
                            nc.tensor.matmul(
                                z01[0:2, 0:512], zsel[:, 2 * r:2 * r + 2],
                                pTs[kt][:],
                                start=(r == 0 and kt == 0),
                                stop=(r == 1 and kt == 3))
                        # AV for this head into shared psc
                        if r == 0:
                            psc = bnd_tile()
                        hh = 2 * e + r
                        for kt in range(4):
                            nc.tensor.matmul(
                                psc[64 * r:64 * r + 64, 0:512],
                                V[kt][:, hh * 64:hh * 64 + 64],
                                pTs[kt][:],
                                start=(kt == 0), stop=(kt == 3))
                    lnz = p_at.tile([2, S], f32, tag="lnz", name="lnz")
                    nc.scalar.activation(lnz[:], z01[0:2, 0:512], AF.Ln)
                    rz01 = p_at.tile([2, S], f32r, tag="z_sb", name="rz01")
                    nc.scalar.activation(rz01[:], lnz[:], AF.Exp, scale=-1.0)
                    zbc = acc_tile(0)
                    nc.tensor.matmul(zbc[:, :], bsel[:], rz01[:],
                                     start=True, stop=True)
                    rzbs = p_at.tile([128, S], bf16, tag="rzs", name="rzbs")
                    nc.scalar.activation(rzbs[:], zbc[:], AF.Copy)
                    cT = p_fm.tile([128, S], bf16, tag=f"qT{e}",
                                   name=f"cT{e}")
                    nc.vector.tensor_mul(cT[:], psc[0:128, 0:512], rzbs[:])
                    ctx_T.append(cT)

                # ---- phase D: O-proj + residual + LN1 ----
                h1, xn1 = [], []
                for half in range(2):
                    ts = (2 * half, 2 * half + 1)
                    pss = {}
                    for ti, t in enumerate(ts):
                        pss[(t, 0)] = acc_tile(2 * ti)
                        pss[(t, 1)] = acc_tile(2 * ti + 1)
                    for k in range(KD):
                        for t in ts:
                            nc.tensor.matmul(
                                pss[(t, 0)][:, 0:384],
                                ctx_T[k][:, t * 128:(t + 1) * 128],
                                wo_sb[:, k * D:k * D + 384],
                                start=(k == 0), stop=(k == KD - 1))
                            nc.tensor.matmul(
                                pss[(t, 1)][:, 0:384],
                                ctx_T[k][:, t * 128:(t + 1) * 128],
                                wo_sb[:, k * D + 384:k * D + 768],
                                start=(k == 0), stop=(k == KD - 1))
                    for t in ts:
                        hp = p_s.tile([128, D], f32, tag="hp", name="hp")
                        nc.vector.tensor_add(hp[:, 0:384],
                                             pss[(t, 0)][:, 0:384],
                                             h[t][:, 0:384])
                        nc.vector.tensor_add(hp[:, 384:768],
                                             pss[(t, 1)][:, 0:384],
                                             h[t][:, 384:768])
                        nc.vector.tensor_add(hp[:], hp[:], bo_bc[:])
                        xnt = p_res.tile([128, D], f32r, tag=f"xn{t}",
                                         name=f"xn1_{t}")
                        h1t = p_res.tile([128, D], f32, tag=f"h{t}",
                                         name=f"h1_{t}")
                        layernorm(hp, xnt, h1t, g1_bc, be1_bc)
                        xn1.append(xnt)
                        h1.append(h1t)

                # ---- phase E: FFN ----
                h1_T = []
                tpv = [acc_tile(i).bitcast(f32r) for i in range(4)]
                tpv.append(bnd_tile()[:, 0:512].bitcast(f32r))
                tpv.append(bnd_tile()[:, 0:512].bitcast(f32r))
                for t in range(NT):
                    for k in range(KD):
                        nc.tensor.matmul(
                            tpv[k][:, t * 128:(t + 1) * 128],
                            xn1[t][:, k * 128:(k + 1) * 128], ident_r[:],
                            is_transpose=True, start=True, stop=True)
                for k in range(KD):
                    hTk = p_fm.tile([128, S], bf16, tag=f"hT{k}",
                                    name=f"h1T{k}")
                    nc.scalar.activation(hTk[:], tpv[k][:], AF.Copy)
                    h1_T.append(hTk)

                for blk in range(4):
                    w1c = p_w.tile([128, 6 * KD * 128], bf16, tag="w1c",
                                   bufs=2, name="w1c")
                    nc.gpsimd.dma_start(
                        w1c[:],
                        w1_p[l, :, blk * 6 * KD * 128:
                             (blk + 1) * 6 * KD * 128])
                    w2c = p_w.tile([128, 6 * D], bf16, tag="w2c",
                                   bufs=2, name="w2c")
                    nc.gpsimd.dma_start(
                        w2c[:], w2_p[l, :, blk * 6 * D:(blk + 1) * 6 * D])
                    g_T = []
                    for j in range(6):
                        i = blk * 6 + j
                        ps = acc_tile(j % 2)
                        for k in range(KD):
                            co = (j * KD + k) * 128
                            nc.tensor.matmul(
                                ps[:], w1c[:, co:co + 128], h1_T[k][:],
                                start=(k == 0), stop=(k == KD - 1))
                        gt = p_fm.tile([128, S], bf16, tag=f"gT{j}",
                                       name=f"gT{j}")
                        nc.scalar.activation(gt[:], ps[:], AF.Gelu,
                                             bias=b1_sb[:, i:i + 1])
                        g_T.append(gt)
                    for half in range(2):
                        ts = (2 * half, 2 * half + 1)
                        pss = {}
                        for ti, t in enumerate(ts):
                            pss[(t, 0)] = acc_tile(2 + ti)
                            pss[(t, 1)] = bnd_tile()
                        for j in range(6):
                            for t in ts:
                                nc.tensor.matmul(
                                    pss[(t, 0)][:, 0:384],
                                    g_T[j][:, t * 128:(t + 1) * 128],
                                    w2c[:, j * D:j * D + 384],
                                    start=(j == 0), stop=(j == 5))
                                nc.tensor.matmul(
                                    pss[(t, 1)][:, 0:384],
                                    g_T[j][:, t * 128:(t + 1) * 128],
                                    w2c[:, j * D + 384:j * D + 768],
                                    start=(j == 0), stop=(j == 5))
                        for t in ts:
                            nc.vector.tensor_add(h1[t][:, 0:384],
                                                 h1[t][:, 0:384],
                                                 pss[(t, 0)][:, 0:384])
                            nc.vector.tensor_add(h1[t][:, 384:768],
                                                 h1[t][:, 384:768],
                                                 pss[(t, 1)][:, 0:384])

                new_h, new_xn = [], []
                for t in range(NT):
                    nc.vector.tensor_add(h1[t][:], h1[t][:], b2_bc[:])
                    xnt = p_res.tile([128, D], f32r, tag=f"xn{t}",
                                     name=f"nxn{t}")
                    ht = p_res.tile([128, D], f32, tag=f"h{t}",
                                    name=f"nh{t}")
                    layernorm(h1[t], xnt, ht, g2_bc, be2_bc)
                    new_h.append(ht)
                    new_xn.append(xnt)
                h, xn = new_h, new_xn

            for t in range(NT):
                nc.sync.dma_start(y[t * 128:(t + 1) * 128, :], h[t][:])

    return nc


def _prep_inputs(inputs):
    fp = np.float32
    b16 = ml_dtypes.bfloat16
    ii = np.ascontiguousarray(inputs["input_ids"], fp)
    am = np.ascontiguousarray(inputs["attn_mask"], fp)
    de = np.asarray(inputs["dist_emb"], fp)
    wq = np.asarray(inputs["wq"], fp)
    wk = np.asarray(inputs["wk"], fp)
    wv = np.asarray(inputs["wv"], fp)
    wo = np.asarray(inputs["wo"], fp)
    w1 = np.asarray(inputs["w1"], fp)
    w2 = np.asarray(inputs["w2"], fp)
    bq = np.asarray(inputs["bq"], fp)
    bk = np.asarray(inputs["bk"], fp)
    bv = np.asarray(inputs["bv"], fp)
    bo = np.asarray(inputs["bo"], fp)
    b1 = np.asarray(inputs["b1"], fp)
    b2 = np.asarray(inputs["b2"], fp)
    g1 = np.asarray(inputs["ln1_g"], fp)
    be1 = np.asarray(inputs["ln1_b"], fp)
    g2 = np.asarray(inputs["ln2_g"], fp)
    be2 = np.asarray(inputs["ln2_b"], fp)
    emb_g = np.asarray(inputs["emb_ln_g"], fp)
    emb_b = np.asarray(inputs["emb_ln_b"], fp)

    # fold the preceding LN affine into Wq/Wk/Wv; fold ln1 into W1
    gprev = np.stack([emb_g] + [g2[ll] for ll in range(L - 1)])
    bprev = np.stack([emb_b] + [be2[ll] for ll in range(L - 1)])
    wqf = gprev[:, :, None] * wq
    wkf = gprev[:, :, None] * wk
    wvf = gprev[:, :, None] * wv
    bqf = bq + np.einsum('ld,lde->le', bprev, wq)
    bkf = bk + np.einsum('ld,lde->le', bprev, wk)
    bvf = bv + np.einsum('ld,lde->le', bprev, wv)
    w1f = g1[:, :, None] * w1
    b1f = b1 + np.einsum('ld,ldi->li', be1, w1)

    def cb(x):
        return np.ascontiguousarray(x.astype(b16))

    shared = dict(
        inw=cb(np.asarray(inputs["in_w"], fp)),
        ttib=cb(np.asarray(inputs["in_b"] + inputs["tte"], fp)),
        embgb=cb(np.concatenate([emb_g, emb_b])),
        wq_p=cb(wqf.reshape(L, KD, 128, KD, 128)
                .transpose(0, 2, 3, 1, 4).reshape(L, 128, KD * KD * 128)),
        wk_p=cb(wkf.reshape(L, KD, 128, KD, 128)
                .transpose(0, 2, 3, 1, 4).reshape(L, 128, KD * KD * 128)),
        wv_p=cb(wvf.reshape(L, KD, 128, D)
                .transpose(0, 2, 1, 3).reshape(L, 128, KD * D)),
        wo_p=cb(wo.reshape(L, KD, 128, D)
                .transpose(0, 2, 1, 3).reshape(L, 128, KD * D)),
        w1_p=cb(w1f.reshape(L, KD, 128, KI, 128)
                .transpose(0, 2, 3, 1, 4).reshape(L, 128, KI * KD * 128)),
        w2_p=cb(w2.reshape(L, KI, 128, D)
                .transpose(0, 2, 1, 3).reshape(L, 128, KI * D)),
        bqc=np.ascontiguousarray(bqf.reshape(L, KD, 128), fp),
        bkc=np.ascontiguousarray(bkf.reshape(L, KD, 128), fp),
        b1c=np.ascontiguousarray(b1f.reshape(L, KI, 128), fp),
        lnpack=cb(np.stack([bvf, bo, b2, g1, be1, g2, be2], axis=1)
                  .reshape(L, 7 * D)),
        de_t=cb(de.transpose(0, 2, 1)),
        de_rt=cb(de[:, ::-1, :].transpose(0, 2, 1)),
        ident_in=cb(np.eye(128, dtype=fp)),
        identr_in=np.eye(128, dtype=fp),
        ones_col_in=cb(np.ones((128, 1), fp)),
        zsel_in=cb(np.stack([np.ones(128), np.zeros(128),
                             np.zeros(128), np.ones(128)], axis=1)),
        bsel_in=np.ascontiguousarray(np.stack([
            np.concatenate([np.ones(64), np.zeros(64)]),
            np.concatenate([np.zeros(64), np.ones(64)])]), fp),
    )
    in_maps = []
    for c in range(B):
        m = dict(shared)
        m["xT"] = cb(ii[c].T)
        m["mask_col"] = np.ascontiguousarray(
            ((1.0 - am[c]) * -1e9)[:, None], fp)
        in_maps.append(m)
    return in_maps


def kernel(trace=False, **inputs):
    if "nc" not in _CACHED:
        _CACHED["nc"] = build_module()
    nc = _CACHED["nc"]
    in_maps = _prep_inputs(inputs)
    res = bass_utils.run_bass_kernel_spmd(
        nc, in_maps, core_ids=list(range(B)), trace=trace)
    out = np.stack([np.asarray(res.results[c]["y"], np.float32)
                    for c in range(B)])
    if trace:
        kernel.last_exec_time_ns = res.exec_time_ns
        kernel.last_results = res
    return out
